# revision 23
# baseline (speedup 1.0000x reference)
"""GATv2 (2-layer, PyG semantics) on 8 Trainium2 NeuronCores.

Strategy (graph/data parallel, dst-sharded), v2:
  - Nodes sharded by destination range across 8 cores (12500 nodes/core).
  - Feature tables have 256B rows [xl(64,f16) | xr(64,f16)] so the batched
    SWDGE ucode gather (dma_gather, int16 idx, 256B elems) applies. Tables
    split into 4 row segments of 25088 (int16 index range); each 128-dst
    window's edge slots are grouped by src-row segment: 4 segments x 5
    tiles of 128 slots. One dma_gather per (window, segment) replaces the
    per-tile indirect gathers: GpSimd desc-gen cost drops from ~19 x 1.4us
    to ~4 x 1.2us + 1 x 1.0us per window.
  - Both layers' tables use the SAME core-major row mapping
    (row = core*12544 + (loc%128)*98 + loc//128), so each window's 128 dst
    rows sit at a fixed stride-98 pattern: the dst-side window fetch is a
    single static DMA with a partition-id dynamic offset (no SWDGE), and
    the two layers share one set of slot/offset index arrays. A transposed
    one-hot (ohT, from a broadcast dst-offset stream) expands xr to edge
    slots on the tensor engine; the xl+xr add rides the same PSUM
    accumulation.
  - Edge softmax math on DVE/ACT; scatter is one-hot matmuls into a
    node-major PSUM aggregate; node-major epilogue; layer-2 tables
    AllGather once; host finishes mean/bias.
"""

import functools
import sys

import numpy as np

sys.path.insert(0, "/opt/trn_rl_repo")

# ---------------------------------------------------------------- constants
N = 100_000
E = 1_600_000
IN = 9
HID = 16
H1 = 4
H2 = 4
OUT = 9
D1 = H1 * HID  # 64
D2 = H2 * OUT  # 36
NEG_ATT = 0.2
NEG_ACT = 0.01
NCORES = 8
NSH = N // NCORES  # 12500 nodes per core
WIN = 128  # dst nodes per window
P = 128
RW = 128  # table row width (f16): [l 64 | r 64] = 256B
SEGS = 4
TSEG = 5  # tiles (of 128 slots) per segment per window
TWS = SEGS * TSEG  # 20 tiles per window
SLOTS = TWS * P  # 2560 slots per window


class Cfg:
    """Compile-time geometry. Full-size defaults; overridable for sim tests."""

    def __init__(self, n=N, e=E, ncores=NCORES, dense_chunks=28, dfc=7):
        self.n = n
        self.e = e
        self.ncores = ncores
        self.nsh = n // ncores
        self.nw = -(-self.nsh // WIN)  # windows per core (98)
        self.rows1 = ((n + 1023) // 1024) * 1024  # 100352
        self.segrows1 = self.rows1 // SEGS  # 25088
        self.rows2sh = self.nw * WIN  # 12544
        self.rows2 = self.rows2sh * ncores  # 100352
        self.segrows2 = self.rows2 // SEGS
        self.dense_tiles = self.rows1 // P  # 784
        self.dense_chunks = dense_chunks
        assert self.dense_tiles % dense_chunks == 0
        self.chunk_tiles = self.dense_tiles // dense_chunks  # 28
        self.wb = 7 if self.chunk_tiles % 7 == 0 else 1
        assert self.chunk_tiles % self.wb == 0
        self.nb = self.dense_tiles // self.wb  # write batches (112)
        self.dfc = dfc  # windows per stream chunk
        assert self.nw % dfc == 0
        self.nwc = self.nw // dfc
        self.iw = SEGS * (TSEG * P // 16)  # idx cols per window (160)
        assert self.segrows1 <= 32768 and self.segrows2 <= 32768


CFG = Cfg()


# ---------------------------------------------------------------- device code
def build_program(cfg: Cfg):
    """Build the SPMD single-core Bass program (same NEFF on all cores)."""
    import concourse.bacc as bacc
    import concourse.bass as bass
    import concourse.tile as tile
    from concourse import mybir

    f16 = mybir.dt.float16
    i32 = mybir.dt.int32
    i16 = mybir.dt.int16
    i8 = mybir.dt.int8
    AF = mybir.ActivationFunctionType
    ALU = mybir.AluOpType

    nc = bacc.Bacc("TRN2", target_bir_lowering=False, debug=False,
                   num_devices=cfg.ncores, num_swdge_queues=4)

    NW = cfg.nw

    # ---------------- dram I/O
    xT = nc.dram_tensor("xT", [IN + 1, cfg.rows1], f16, kind="ExternalInput")
    w1c = nc.dram_tensor("w1c", [IN + 1, RW], f16, kind="ExternalInput")
    w2c = nc.dram_tensor("w2c", [D1 + 1, RW], f16, kind="ExternalInput")
    f32_ = mybir.dt.float32
    att1 = nc.dram_tensor("att1", [1, D1], f16, kind="ExternalInput")
    att2 = nc.dram_tensor("att2", [1, D2], f16, kind="ExternalInput")
    bias1r = nc.dram_tensor("bias1r", [1, D1], f32_, kind="ExternalInput")
    slw = nc.dram_tensor("slw", [P, NW * cfg.iw], i16, kind="ExternalInput")
    doffd = nc.dram_tensor("doffd", [P, NW * TWS], i8, kind="ExternalInput")
    dofft = nc.dram_tensor("dofft", [NW, SLOTS], i8, kind="ExternalInput")
    out_raw = nc.dram_tensor("out_raw", [cfg.rows2sh, D2], f32_,
                             kind="ExternalOutput")

    t1g = nc.dram_tensor("t1g", [cfg.rows1, RW], f16)
    t2sh = nc.dram_tensor("t2sh", [cfg.rows2sh, RW], f16)
    t2g = nc.dram_tensor("t2g", [cfg.rows2, RW], f16, addr_space="Shared")

    OC1 = D1 + H1  # agg cols layer 1 (num 64 + den 4)
    OC2 = D2 + H2  # agg cols layer 2 (num 36 + den 4)

    with tile.TileContext(nc) as tc:
        import contextlib
        ctx = contextlib.ExitStack()
        with ctx:
            consts = ctx.enter_context(tc.tile_pool(name="consts", bufs=1))
            idxp = ctx.enter_context(tc.tile_pool(name="idxp", bufs=1))
            idxs = ctx.enter_context(tc.tile_pool(name="idxs", bufs=2))
            xtp = ctx.enter_context(tc.tile_pool(name="xtp", bufs=2))
            stage = ctx.enter_context(tc.tile_pool(name="stage", bufs=3))
            gath = ctx.enter_context(tc.tile_pool(name="gath", bufs=2))
            ohp = ctx.enter_context(tc.tile_pool(name="ohp", bufs=2))
            emath = ctx.enter_context(tc.tile_pool(name="emath", bufs=2))
            wtp = ctx.enter_context(tc.tile_pool(name="wtp", bufs=2))
            epi = ctx.enter_context(tc.tile_pool(name="epi", bufs=2))
            # ---------------- constants into SBUF
            w1c_sb = consts.tile([IN + 1, RW], f16)
            nc.sync.dma_start(out=w1c_sb[:], in_=w1c.ap())
            w2c_sb = consts.tile([D1 + 1, RW], f16)
            nc.sync.dma_start(out=w2c_sb[:], in_=w2c.ap())
            b1rep = consts.tile([P, D1], f32_)
            nc.sync.dma_start(out=b1rep[0:1, :], in_=bias1r.ap())
            nc.gpsimd.partition_broadcast(b1rep[:], b1rep[0:1, :])
            att1_sb = consts.tile([P, D1], f16)
            nc.sync.dma_start(out=att1_sb[0:1, :], in_=att1.ap())
            nc.gpsimd.partition_broadcast(att1_sb[:], att1_sb[0:1, :])
            att2_sb = consts.tile([P, D2], f16)
            nc.sync.dma_start(out=att2_sb[0:1, :], in_=att2.ap())
            nc.gpsimd.partition_broadcast(att2_sb[:], att2_sb[0:1, :])
            iota_i = consts.tile([P, P], i32)
            nc.gpsimd.iota(iota_i[:], pattern=[[1, P]], base=0,
                           channel_multiplier=0)
            iota_f = consts.tile([P, P], f16)
            nc.vector.tensor_copy(out=iota_f[:], in_=iota_i[:])
            iotac_i = consts.tile([P, 1], i32)
            nc.gpsimd.iota(iotac_i[:], pattern=[[0, 1]], base=0,
                           channel_multiplier=1)
            iotac_f = consts.tile([P, 1], f32_)
            nc.vector.tensor_copy(out=iotac_f[:], in_=iotac_i[:])
            ident = consts.tile([P, P], f16)
            nc.vector.tensor_scalar(ident[:], iota_f[:], iotac_f[:], None,
                                    op0=ALU.is_equal)
            epsP = consts.tile([P, 1], f32_)
            nc.vector.memset(epsP[:], 1e-16)
            iota_b = consts.tile([P, P], i8)
            nc.vector.tensor_copy(out=iota_b[:], in_=iota_i[:])
            iotac_b = consts.tile([P, 1], i8)
            nc.vector.tensor_copy(out=iotac_b[:], in_=iotac_i[:])

            # small idx arrays, resident
            doff_sb = idxp.tile([P, NW * TWS], i8)
            nc.sync.dma_start(out=doff_sb[:], in_=doffd.ap())

            # ---------------- phase 1: dense layer-1 table
            # xT is core-major padded (12544 cols/core); node (k, loc) sits
            # at row k*12544 + (loc%128)*98 + loc//128, so one write batch
            # (wb=7 node-tiles) is 7*256B contiguous per partition.
            ck = cfg.chunk_tiles
            nqb = cfg.nw // cfg.wb
            t1v = t1g.ap().rearrange(
                "(c p qb qw) f -> c qb p (qw f)", c=cfg.ncores, p=P,
                qb=nqb, qw=cfg.wb)
            with tc.tile_pool(name="mmp", bufs=4, space="PSUM") as mmp:
                for c in range(cfg.dense_chunks):
                    xt_sb = xtp.tile([IN + 1, ck * P], f16)
                    nc.sync.dma_start(
                        out=xt_sb[:],
                        in_=xT.ap()[:, c * ck * P:(c + 1) * ck * P])
                    for b in range(ck // cfg.wb):
                        st = stage.tile([P, cfg.wb, RW], f16)
                        for j in range(cfg.wb):
                            t = b * cfg.wb + j
                            mm = mmp.tile([P, RW], f32_)
                            nc.tensor.matmul(
                                out=mm[:], lhsT=xt_sb[:, t * P:(t + 1) * P],
                                rhs=w1c_sb[:], start=True, stop=True)
                            if j % 2 == 0:
                                nc.scalar.copy(out=st[:, j, :], in_=mm[:])
                            else:
                                nc.vector.tensor_copy(out=st[:, j, :],
                                                      in_=mm[:])
                        gb = c * (ck // cfg.wb) + b
                        nc.sync.dma_start(
                            out=t1v[gb // nqb, gb % nqb],
                            in_=st[:].rearrange("p t f -> p (t f)"))

            attrep1 = consts.tile([P, TWS, D1], f16)
            nc.vector.tensor_copy(
                out=attrep1[:],
                in_=att1_sb[:, 0:D1].unsqueeze(1).to_broadcast(
                    [P, TWS, D1]))
            attrep2 = consts.tile([P, TWS, D2], f16)
            nc.vector.tensor_copy(
                out=attrep2[:],
                in_=att2_sb[:, 0:D2].unsqueeze(1).to_broadcast(
                    [P, TWS, D2]))
            pid = nc.partition_id()
            tc.strict_bb_all_engine_barrier()

            xrep = ctx.enter_context(tc.tile_pool(name="xrep", bufs=2,
                                                  space="PSUM"))
            aggp = ctx.enter_context(tc.tile_pool(name="aggp", bufs=2,
                                                  space="PSUM"))

            # ---------------- edge phase builder (shared by both layers)
            def edge_layer(layer):
                if layer == 1:
                    D, H, C, OC = D1, H1, HID, OC1
                    table, segrows, attrep = t1g, cfg.segrows1, attrep1
                else:
                    D, H, C, OC = D2, H2, OUT, OC2
                    table, segrows, attrep = t2g, cfg.segrows2, attrep2

                t2v = t2sh.ap().rearrange("(p w) f -> w p f", p=P, w=NW)
                # window dst rows: c*12544 + p*98 + w -> static strided DMA
                tv = table.ap().rearrange("(cp q) f -> cp q f", q=NW)

                for wc in range(cfg.nwc):
                    # stream this chunk's gather indices
                    sl_sb = idxs.tile([P, cfg.dfc * cfg.iw], i16, tag="sl")
                    nc.sync.dma_start(
                        out=sl_sb[:],
                        in_=slw.ap()[:, wc * cfg.dfc * cfg.iw:
                                     (wc + 1) * cfg.dfc * cfg.iw])
                    for wi in range(cfg.dfc):
                        w = wc * cfg.dfc + wi
                        # --- dst-offset row broadcast (HWDGE, static slice)
                        dft = gath.tile([P, TWS, P], i8, tag="dft", bufs=3)
                        nc.sync.dma_start(
                            out=dft[:].rearrange("p t e -> p (t e)"),
                            in_=dofft.ap()[w:w + 1, :].to_broadcast(
                                [P, SLOTS]))
                        # --- segmented src gathers (SWDGE ucode)
                        xg = gath.tile([P, TWS, RW], f16, tag="xg", bufs=3)
                        iw0 = wi * cfg.iw
                        for s in range(SEGS):
                            nc.gpsimd.dma_gather(
                                out_ap=xg[:, s * TSEG:(s + 1) * TSEG, :],
                                in_ap=table.ap()[s * segrows:
                                                 (s + 1) * segrows, :],
                                idxs_ap=sl_sb[:, iw0 + s * (TSEG * P // 16):
                                              iw0 + (s + 1) * (TSEG * P // 16)],
                                num_idxs=TSEG * P, num_idxs_reg=TSEG * P,
                                elem_size=RW, queue_num=s)
                        # --- window dst rows: static DMA at pid offset
                        xrw = gath.tile([P, RW], f16, tag="xrw", bufs=3)
                        nc.sync.dma_start(
                            out=xrw[:],
                            in_=tv[bass.ds(pid * P, P), w, :])

                        # --- one-hots (DVE)
                        oh = ohp.tile([P, TWS, P], f16, tag="oh")
                        nc.vector.tensor_tensor(
                            out=oh[:],
                            in0=iota_b[:].unsqueeze(1).to_broadcast(
                                [P, TWS, P]),
                            in1=doff_sb[:, w * TWS:(w + 1) * TWS]
                                .unsqueeze(2).to_broadcast([P, TWS, P]),
                            op=ALU.is_equal)
                        ohT = ohp.tile([P, TWS, P], f16, tag="ohT")
                        nc.vector.tensor_scalar(ohT[:], dft[:], iotac_f[:],
                                                None, op0=ALU.is_equal)

                        # --- contiguous xl copy (ACT); frees xg early
                        xls = emath.tile([P, TWS, D], f16, tag="xls",
                                         bufs=3)
                        nc.scalar.copy(out=xls[:], in_=xg[:, :, 0:D])

                        # --- epre = ohT@xr + I@xl accumulated on PE (PSUM)
                        xre = xrep.tile([P, TWS, D], f32_, tag="xre")
                        for t in range(TWS):
                            nc.tensor.matmul(out=xre[:, t, :],
                                             lhsT=ohT[:, t, :],
                                             rhs=xrw[:, D1:D1 + D],
                                             start=True, stop=False)
                            nc.tensor.matmul(out=xre[:, t, :],
                                             lhsT=ident[:],
                                             rhs=xls[:, t, :],
                                             start=False, stop=True)

                        # --- edge softmax math (leaky on ACT, reads PSUM)
                        ee = emath.tile([P, TWS, D], f16, tag="ee", bufs=3)
                        nc.scalar.activation(out=ee[:], in_=xre[:],
                                             func=AF.Prelu, alpha=NEG_ATT)
                        tmp = emath.tile([P, TWS, D], f16, tag="tmp", bufs=3)
                        nc.vector.tensor_tensor(
                            out=tmp[:], in0=ee[:], in1=attrep[:],
                            op=ALU.mult)
                        logits = emath.tile([P, TWS * H], f16, tag="logits")
                        with nc.allow_low_precision(
                                reason="9-16 term f16 logit sums"):
                            nc.vector.tensor_reduce(
                                out=logits[:],
                                in_=tmp[:].rearrange(
                                    "p t (h c) -> p (t h) c", c=C),
                                axis=mybir.AxisListType.X, op=ALU.add)

                        wtx = wtp.tile([P, TWS, D], f16, tag="wtx", bufs=3)
                        nc.scalar.activation(
                            out=wtx[:].rearrange("p t (h c) -> p t h c",
                                                 c=C),
                            in_=logits[:].rearrange("p (t h) -> p t h", h=H)
                            .unsqueeze(3).to_broadcast([P, TWS, H, C]),
                            func=AF.Exp)
                        wt = wtp.tile([P, TWS, OC], f16, tag="wt", bufs=3)
                        nc.scalar.activation(
                            out=wt[:, :, D:OC],
                            in_=logits[:].rearrange("p (t h) -> p t h", h=H),
                            func=AF.Exp)
                        nc.vector.tensor_tensor(out=wt[:, :, 0:D],
                                                in0=xls[:],
                                                in1=wtx[:], op=ALU.mult)

                        # --- scatter into node-major aggregate (PE)
                        # one PSUM bank shared: aggT | t2p | hTp slices
                        bank = aggp.tile([P, 512], f32_, tag="bank")
                        aggT = bank[:, 0:OC]
                        for t in range(TWS):
                            nc.tensor.matmul(out=aggT[:], lhsT=oh[:, t, :],
                                             rhs=wt[:, t, :], start=(t == 0),
                                             stop=(t == TWS - 1))

                        # --- epilogue (node-major)
                        den = epi.tile([P, H], f32_, tag="den")
                        nc.vector.tensor_scalar(den[:], aggT[:, D:OC],
                                                epsP[:], None, op0=ALU.add)
                        rec = epi.tile([P, H], f32_, tag="rec")
                        nc.vector.reciprocal(out=rec[:], in_=den[:])
                        hpre = epi.tile([P, D], f32_, tag="hpre")
                        nc.vector.tensor_tensor(
                            out=hpre[:].rearrange("p (h c) -> p h c", c=C),
                            in0=aggT[:, 0:D].rearrange("p (h c) -> p h c",
                                                       c=C),
                            in1=rec[:].unsqueeze(2).to_broadcast([P, H, C]),
                            op=ALU.mult)

                        if layer == 1:
                            hb = epi.tile([P, D1], f32_, tag="hb")
                            nc.vector.tensor_tensor(out=hb[:], in0=hpre[:],
                                                    in1=b1rep[:], op=ALU.add)
                            h = epi.tile([P, D1], f16, tag="h")
                            nc.scalar.activation(out=h[:], in_=hb[:],
                                                 func=AF.Prelu, alpha=NEG_ACT)
                            hTp = bank[0:D1, 384:512]
                            nc.tensor.matmul(out=hTp[:], lhsT=h[:],
                                             rhs=ident[:], start=True,
                                             stop=True)
                            hT = epi.tile([D1 + 1, P], f16, tag="hT")
                            nc.scalar.copy(out=hT[0:D1, :], in_=hTp[:])
                            nc.vector.memset(hT[D1:D1 + 1, :], 1.0)
                            t2p = bank[:, 128:256]
                            nc.tensor.matmul(out=t2p[:], lhsT=hT[:],
                                             rhs=w2c_sb[:], start=True,
                                             stop=True)
                            st2 = epi.tile([P, RW], f16, tag="st2")
                            nc.scalar.copy(out=st2[:], in_=t2p[:])
                            nc.sync.dma_start(out=t2v[w], in_=st2[:])
                        else:
                            nc.sync.dma_start(
                                out=out_raw.ap()[w * P:(w + 1) * P, :],
                                in_=hpre[:])

            # ---------------- phase 2: layer-1 edges
            edge_layer(1)
            tc.strict_bb_all_engine_barrier()

            # ---------------- phase 3: allgather layer-2 table
            nc.gpsimd.collective_compute(
                "AllGather", mybir.AluOpType.bypass,
                replica_groups=[list(range(cfg.ncores))],
                ins=[t2sh.ap()], outs=[t2g.ap()])
            tc.strict_bb_all_engine_barrier()

            # ---------------- phase 4: layer-2 edges
            edge_layer(2)

    nc.compile()
    return nc


# ---------------------------------------------------------------- host prep
def host_prep(x, edge_index, W1l, b1l, W1r, b1r, att1, bias1,
              W2l, b2l, W2r, b2r, att2, bias2, cfg: Cfg):
    """Numpy-only preprocessing: edge sort/segment/pad + weight layouts."""
    n, e, nsh = cfg.n, cfg.e, cfg.nsh
    src = np.asarray(edge_index[0], dtype=np.int64)
    dst = np.asarray(edge_index[1], dtype=np.int64)

    order = np.argsort(dst, kind="stable")
    src_s, dst_s = src[order], dst[order]
    bounds = np.searchsorted(dst_s, np.arange(cfg.ncores + 1) * nsh)

    def rowmap(g):
        k, loc = g // nsh, g % nsh
        return k * cfg.rows2sh + (loc % P) * cfg.nw + loc // P

    nslots = cfg.nw * SLOTS
    segslots = TSEG * P  # 640
    segrows = cfg.segrows1

    per_core = []
    for k in range(cfg.ncores):
        sl = slice(bounds[k], bounds[k + 1])
        sk, dk = src_s[sl], dst_s[sl]
        dloc = dk - k * nsh
        win = dloc >> 7
        r = rowmap(sk)
        seg = r // segrows
        key = win * SEGS + seg
        o2 = np.argsort(key, kind="stable")
        ks, rs, dls = key[o2], r[o2], dloc[o2]
        gb = np.searchsorted(ks, np.arange(cfg.nw * SEGS + 1))
        cnt = np.diff(gb)
        if cnt.max(initial=0) > segslots:
            return None  # static schedule overflow -> caller falls back
        pos = (ks * segslots + np.arange(len(ks)) - gb[ks]).astype(np.int64)
        idxf = np.zeros(nslots, np.int16)
        off = np.full(nslots, -1.0, np.float16)
        idxf[pos] = (rs - (ks % SEGS) * segrows).astype(np.int16)
        off[pos] = (dls - (ks // SEGS) * WIN).astype(np.float16)

        # wrap idx per (window,segment): [16, 40] blocks, replicate x8
        blocks = idxf.reshape(cfg.nw * SEGS, segslots // 16, 16)
        blocks = blocks.transpose(0, 2, 1)  # [NW*SEGS, 16, 40]
        slw16 = np.ascontiguousarray(
            blocks.transpose(1, 0, 2).reshape(16, -1))  # [16, NW*160]
        core = dict(
            slw=np.tile(slw16, (8, 1)).astype(np.int16),
            doffd=np.ascontiguousarray(
                off.reshape(cfg.nw * TWS, P).T).astype(np.int8),
            dofft=np.ascontiguousarray(
                off.reshape(cfg.nw, SLOTS)).astype(np.int8))
        per_core.append(core)

    # xT core-major padded: node (k, loc) at column k*12544 + loc
    xTh = np.zeros((IN + 1, cfg.rows1), np.float16)
    xTh[IN, :] = 1.0
    g = np.arange(n)
    cols = (g // nsh) * cfg.rows2sh + (g % nsh)
    xTh[:IN, cols] = np.asarray(x, np.float32).T.astype(np.float16)
    w1cat = np.concatenate([np.asarray(W1l), np.asarray(W1r)], axis=0)
    w1b = np.concatenate([np.asarray(b1l), np.asarray(b1r)])[None, :]
    w1c_h = np.concatenate([w1cat.T, w1b], axis=0).astype(np.float16)
    w2c_h = np.zeros((D1 + 1, RW), np.float16)
    w2c_h[:D1, 0:D2] = np.asarray(W2l).T
    w2c_h[:D1, D1:D1 + D2] = np.asarray(W2r).T
    w2c_h[D1, 0:D2] = np.asarray(b2l)
    w2c_h[D1, D1:D1 + D2] = np.asarray(b2r)
    att1_h = np.asarray(att1, np.float32).reshape(1, D1).astype(np.float16)
    att2_h = np.asarray(att2, np.float32).reshape(1, D2).astype(np.float16)
    bias1_h = np.asarray(bias1, np.float32).reshape(1, D1)

    shared = dict(xT=xTh, w1c=w1c_h, w2c=w2c_h, att1=att1_h,
                  att2=att2_h, bias1r=bias1_h)
    in_maps = [dict(shared, **pc) for pc in per_core]
    return in_maps


def assemble_output(results, bias2, cfg: Cfg):
    outs = []
    b2 = np.asarray(bias2, np.float32)
    for k in range(cfg.ncores):
        arr = results[k]["out_raw"][:cfg.nsh]  # [nsh, 36] node-major
        outk = arr.reshape(cfg.nsh, H2, OUT).mean(axis=1) + b2[None, :]
        outs.append(outk.astype(np.float32))
    return np.concatenate(outs, axis=0)


# ---------------------------------------------------------------- fallback
def _reference_numpy(x, edge_index, W1l, b1l, W1r, b1r, att1, bias1,
                     W2l, b2l, W2r, b2r, att2, bias2):
    """Pure-numpy fallback (used only if inputs don't fit the static plan)."""
    def gatv2(x, src, dst, Wl, bl, Wr, br, att, bias, concat):
        n = x.shape[0]
        H, C = att.shape
        xl = (x @ Wl.T + bl).reshape(n, H, C)
        xr = (x @ Wr.T + br).reshape(n, H, C)
        ee = xl[src] + xr[dst]
        ee = np.where(ee > 0, ee, NEG_ATT * ee)
        logits = np.einsum("ehc,hc->eh", ee, att)
        m = np.full((n, H), -np.inf, np.float32)
        np.maximum.at(m, dst, logits)
        m = np.where(np.isfinite(m), m, 0.0)
        p = np.exp(logits - m[dst])
        den = np.zeros((n, H), np.float32)
        np.add.at(den, dst, p)
        alpha = p / (den[dst] + 1e-16)
        out = np.zeros((n, H, C), np.float32)
        np.add.at(out, dst, alpha[..., None] * xl[src])
        if concat:
            return out.reshape(n, H * C) + bias
        return out.mean(axis=1) + bias

    src, dst = edge_index[0].astype(np.int64), edge_index[1].astype(np.int64)
    h = gatv2(np.asarray(x, np.float32), src, dst, W1l, b1l, W1r, b1r, att1,
              bias1, True)
    h = np.where(h > 0, h, NEG_ACT * h)
    return gatv2(h, src, dst, W2l, b2l, W2r, b2r, att2, bias2, False)


# ---------------------------------------------------------------- entry point
@functools.lru_cache(maxsize=1)
def _compiled():
    return build_program(CFG)


_LAST_RESULTS = {}


def kernel(x, edge_index, W1l, b1l, W1r, b1r, att1, bias1,
           W2l, b2l, W2r, b2r, att2, bias2):
    args = (x, edge_index, W1l, b1l, W1r, b1r, att1, bias1,
            W2l, b2l, W2r, b2r, att2, bias2)
    if (np.asarray(x).shape != (N, IN)
            or np.asarray(edge_index).shape != (2, E)):
        return _reference_numpy(*[np.asarray(a, np.float32) if i != 1 else
                                  np.asarray(a) for i, a in enumerate(args)])

    in_maps = host_prep(*args, CFG)
    if in_maps is None:
        return _reference_numpy(*[np.asarray(a, np.float32) if i != 1 else
                                  np.asarray(a) for i, a in enumerate(args)])

    from concourse.bass_utils import run_bass_kernel_spmd
    nc = _compiled()
    res = run_bass_kernel_spmd(nc, in_maps, core_ids=list(range(NCORES)),
                               trace=False)
    _LAST_RESULTS["res"] = res
    return assemble_output(res.results, bias2, CFG)


# revision 24
# speedup vs baseline: 1.1478x; 1.1478x over previous
"""GATv2 (2-layer, PyG semantics) on 8 Trainium2 NeuronCores.

Strategy (graph/data parallel, dst-sharded), v2:
  - Nodes sharded by destination range across 8 cores (12500 nodes/core).
  - Feature tables have 256B rows [xl(64,f16) | xr(64,f16)] so the batched
    SWDGE ucode gather (dma_gather, int16 idx, 256B elems) applies. Tables
    split into 4 row segments of 25088 (int16 index range); each 128-dst
    window's edge slots are grouped by src-row segment: 4 segments x 5
    tiles of 128 slots. One dma_gather per (window, segment) replaces the
    per-tile indirect gathers: GpSimd desc-gen cost drops from ~19 x 1.4us
    to ~4 x 1.2us + 1 x 1.0us per window.
  - Both layers' tables use the SAME core-major row mapping
    (row = core*12544 + (loc%128)*98 + loc//128), so each window's 128 dst
    rows sit at a fixed stride-98 pattern: the dst-side window fetch is a
    single static DMA with a partition-id dynamic offset (no SWDGE), and
    the two layers share one set of slot/offset index arrays. A transposed
    one-hot (ohT, from a broadcast dst-offset stream) expands xr to edge
    slots on the tensor engine; the xl+xr add rides the same PSUM
    accumulation.
  - Edge softmax math on DVE/ACT; scatter is one-hot matmuls into a
    node-major PSUM aggregate; node-major epilogue; layer-2 tables
    AllGather once; host finishes mean/bias.
"""

import functools
import sys

import numpy as np

sys.path.insert(0, "/opt/trn_rl_repo")

# ---------------------------------------------------------------- constants
N = 100_000
E = 1_600_000
IN = 9
HID = 16
H1 = 4
H2 = 4
OUT = 9
D1 = H1 * HID  # 64
D2 = H2 * OUT  # 36
NEG_ATT = 0.2
NEG_ACT = 0.01
NCORES = 8
NSH = N // NCORES  # 12500 nodes per core
WIN = 128  # dst nodes per window
P = 128
RW = 128  # table row width (f16): [l 64 | r 64] = 256B
SEGS = 4
TSEG = 5  # tiles (of 128 slots) per segment per window
TWS = SEGS * TSEG  # 20 tiles per window
SLOTS = TWS * P  # 2560 slots per window


class Cfg:
    """Compile-time geometry. Full-size defaults; overridable for sim tests."""

    def __init__(self, n=N, e=E, ncores=NCORES, dense_chunks=28, dfc=7):
        self.n = n
        self.e = e
        self.ncores = ncores
        self.nsh = n // ncores
        self.nw = -(-self.nsh // WIN)  # windows per core (98)
        self.rows1 = ((n + 1023) // 1024) * 1024  # 100352
        self.segrows1 = self.rows1 // SEGS  # 25088
        self.rows2sh = self.nw * WIN  # 12544
        self.rows2 = self.rows2sh * ncores  # 100352
        self.segrows2 = self.rows2 // SEGS
        self.dense_tiles = self.rows1 // P  # 784
        self.dense_chunks = dense_chunks
        assert self.dense_tiles % dense_chunks == 0
        self.chunk_tiles = self.dense_tiles // dense_chunks  # 28
        self.wb = 7 if self.chunk_tiles % 7 == 0 else 1
        assert self.chunk_tiles % self.wb == 0
        self.nb = self.dense_tiles // self.wb  # write batches (112)
        self.dfc = dfc  # windows per stream chunk
        assert self.nw % dfc == 0
        self.nwc = self.nw // dfc
        self.iw = SEGS * (TSEG * P // 16)  # idx cols per window (160)
        assert self.segrows1 <= 32768 and self.segrows2 <= 32768


CFG = Cfg()


# ---------------------------------------------------------------- device code
def build_program(cfg: Cfg):
    """Build the SPMD single-core Bass program (same NEFF on all cores)."""
    import concourse.bacc as bacc
    import concourse.bass as bass
    import concourse.tile as tile
    from concourse import mybir

    f16 = mybir.dt.float16
    i32 = mybir.dt.int32
    i16 = mybir.dt.int16
    i8 = mybir.dt.int8
    AF = mybir.ActivationFunctionType
    ALU = mybir.AluOpType

    nc = bacc.Bacc("TRN2", target_bir_lowering=False, debug=False,
                   num_devices=cfg.ncores, num_swdge_queues=4)

    NW = cfg.nw

    # ---------------- dram I/O
    xT = nc.dram_tensor("xT", [IN + 1, cfg.rows1], f16, kind="ExternalInput")
    w1c = nc.dram_tensor("w1c", [IN + 1, RW], f16, kind="ExternalInput")
    w2c = nc.dram_tensor("w2c", [D1 + 1, RW], f16, kind="ExternalInput")
    f32_ = mybir.dt.float32
    att1 = nc.dram_tensor("att1", [1, D1], f16, kind="ExternalInput")
    att2 = nc.dram_tensor("att2", [1, D2], f16, kind="ExternalInput")
    bias1r = nc.dram_tensor("bias1r", [1, D1], f32_, kind="ExternalInput")
    slw = nc.dram_tensor("slw", [P, NW * cfg.iw], i16, kind="ExternalInput")
    doffd = nc.dram_tensor("doffd", [P, NW * TWS], f16, kind="ExternalInput")
    dofft = nc.dram_tensor("dofft", [NW, SLOTS], i8, kind="ExternalInput")
    out_raw = nc.dram_tensor("out_raw", [cfg.rows2sh, D2], f32_,
                             kind="ExternalOutput")

    t1g = nc.dram_tensor("t1g", [cfg.rows1, RW], f16)
    t2sh = nc.dram_tensor("t2sh", [cfg.rows2sh, RW], f16)
    t2g = nc.dram_tensor("t2g", [cfg.rows2, RW], f16, addr_space="Shared")

    OC1 = D1 + H1  # agg cols layer 1 (num 64 + den 4)
    OC2 = D2 + H2  # agg cols layer 2 (num 36 + den 4)

    with tile.TileContext(nc) as tc:
        import contextlib
        ctx = contextlib.ExitStack()
        with ctx:
            consts = ctx.enter_context(tc.tile_pool(name="consts", bufs=1))
            idxp = ctx.enter_context(tc.tile_pool(name="idxp", bufs=1))
            idxs = ctx.enter_context(tc.tile_pool(name="idxs", bufs=2))
            xtp = ctx.enter_context(tc.tile_pool(name="xtp", bufs=2))
            stage = ctx.enter_context(tc.tile_pool(name="stage", bufs=3))
            gath = ctx.enter_context(tc.tile_pool(name="gath", bufs=2))
            ohp = ctx.enter_context(tc.tile_pool(name="ohp", bufs=2))
            emath = ctx.enter_context(tc.tile_pool(name="emath", bufs=2))
            wtp = ctx.enter_context(tc.tile_pool(name="wtp", bufs=2))
            epi = ctx.enter_context(tc.tile_pool(name="epi", bufs=2))
            # ---------------- constants into SBUF
            w1c_sb = consts.tile([IN + 1, RW], f16)
            nc.sync.dma_start(out=w1c_sb[:], in_=w1c.ap())
            w2c_sb = consts.tile([D1 + 1, RW], f16)
            nc.sync.dma_start(out=w2c_sb[:], in_=w2c.ap())
            b1rep = consts.tile([P, D1], f32_)
            nc.sync.dma_start(out=b1rep[0:1, :], in_=bias1r.ap())
            nc.gpsimd.partition_broadcast(b1rep[:], b1rep[0:1, :])
            att1_sb = consts.tile([P, D1], f16)
            nc.sync.dma_start(out=att1_sb[0:1, :], in_=att1.ap())
            nc.gpsimd.partition_broadcast(att1_sb[:], att1_sb[0:1, :])
            att2_sb = consts.tile([P, D2], f16)
            nc.sync.dma_start(out=att2_sb[0:1, :], in_=att2.ap())
            nc.gpsimd.partition_broadcast(att2_sb[:], att2_sb[0:1, :])
            iota_i = consts.tile([P, P], i32)
            nc.gpsimd.iota(iota_i[:], pattern=[[1, P]], base=0,
                           channel_multiplier=0)
            iota_f = consts.tile([P, P], f16)
            nc.vector.tensor_copy(out=iota_f[:], in_=iota_i[:])
            iotac_i = consts.tile([P, 1], i32)
            nc.gpsimd.iota(iotac_i[:], pattern=[[0, 1]], base=0,
                           channel_multiplier=1)
            iotac_f = consts.tile([P, 1], f32_)
            nc.vector.tensor_copy(out=iotac_f[:], in_=iotac_i[:])
            ident = consts.tile([P, P], f16)
            nc.vector.tensor_scalar(ident[:], iota_f[:], iotac_f[:], None,
                                    op0=ALU.is_equal)
            epsP = consts.tile([P, 1], f32_)
            nc.vector.memset(epsP[:], 1e-16)

            # small idx arrays, resident
            doff_sb = idxp.tile([P, NW * TWS], f16)
            nc.sync.dma_start(out=doff_sb[:], in_=doffd.ap())

            # ---------------- phase 1: dense layer-1 table
            # xT is core-major padded (12544 cols/core); node (k, loc) sits
            # at row k*12544 + (loc%128)*98 + loc//128, so one write batch
            # (wb=7 node-tiles) is 7*256B contiguous per partition.
            ck = cfg.chunk_tiles
            nqb = cfg.nw // cfg.wb
            t1v = t1g.ap().rearrange(
                "(c p qb qw) f -> c qb p (qw f)", c=cfg.ncores, p=P,
                qb=nqb, qw=cfg.wb)
            with tc.tile_pool(name="mmp", bufs=4, space="PSUM") as mmp:
                for c in range(cfg.dense_chunks):
                    xt_sb = xtp.tile([IN + 1, ck * P], f16)
                    nc.sync.dma_start(
                        out=xt_sb[:],
                        in_=xT.ap()[:, c * ck * P:(c + 1) * ck * P])
                    for b in range(ck // cfg.wb):
                        st = stage.tile([P, cfg.wb, RW], f16)
                        for j in range(cfg.wb):
                            t = b * cfg.wb + j
                            mm = mmp.tile([P, RW], f32_)
                            nc.tensor.matmul(
                                out=mm[:], lhsT=xt_sb[:, t * P:(t + 1) * P],
                                rhs=w1c_sb[:], start=True, stop=True)
                            if j % 2 == 0:
                                nc.scalar.copy(out=st[:, j, :], in_=mm[:])
                            else:
                                nc.vector.tensor_copy(out=st[:, j, :],
                                                      in_=mm[:])
                        gb = c * (ck // cfg.wb) + b
                        nc.sync.dma_start(
                            out=t1v[gb // nqb, gb % nqb],
                            in_=st[:].rearrange("p t f -> p (t f)"))

            attrep1 = consts.tile([P, TWS, D1], f16)
            nc.vector.tensor_copy(
                out=attrep1[:],
                in_=att1_sb[:, 0:D1].unsqueeze(1).to_broadcast(
                    [P, TWS, D1]))
            attrep2 = consts.tile([P, TWS, D2], f16)
            nc.vector.tensor_copy(
                out=attrep2[:],
                in_=att2_sb[:, 0:D2].unsqueeze(1).to_broadcast(
                    [P, TWS, D2]))
            pid = nc.partition_id()
            tc.strict_bb_all_engine_barrier()

            xrep = ctx.enter_context(tc.tile_pool(name="xrep", bufs=2,
                                                  space="PSUM"))
            aggp = ctx.enter_context(tc.tile_pool(name="aggp", bufs=2,
                                                  space="PSUM"))

            # ---------------- edge phase builder (shared by both layers)
            def edge_layer(layer):
                if layer == 1:
                    D, H, C, OC = D1, H1, HID, OC1
                    table, segrows, attrep = t1g, cfg.segrows1, attrep1
                else:
                    D, H, C, OC = D2, H2, OUT, OC2
                    table, segrows, attrep = t2g, cfg.segrows2, attrep2

                t2v = t2sh.ap().rearrange("(p w) f -> w p f", p=P, w=NW)
                # window dst rows: c*12544 + p*98 + w -> static strided DMA
                tv = table.ap().rearrange("(cp q) f -> cp q f", q=NW)

                for wc in range(cfg.nwc):
                    # stream this chunk's gather indices
                    sl_sb = idxs.tile([P, cfg.dfc * cfg.iw], i16, tag="sl")
                    nc.sync.dma_start(
                        out=sl_sb[:],
                        in_=slw.ap()[:, wc * cfg.dfc * cfg.iw:
                                     (wc + 1) * cfg.dfc * cfg.iw])
                    for wi in range(cfg.dfc):
                        w = wc * cfg.dfc + wi
                        # --- dst-offset row broadcast (HWDGE, static slice)
                        dft = gath.tile([P, TWS, P], i8, tag="dft", bufs=3)
                        nc.sync.dma_start(
                            out=dft[:].rearrange("p t e -> p (t e)"),
                            in_=dofft.ap()[w:w + 1, :].to_broadcast(
                                [P, SLOTS]))
                        # --- segmented src gathers (SWDGE ucode)
                        xg = gath.tile([P, TWS, RW], f16, tag="xg", bufs=3)
                        iw0 = wi * cfg.iw
                        for s in range(SEGS):
                            nc.gpsimd.dma_gather(
                                out_ap=xg[:, s * TSEG:(s + 1) * TSEG, :],
                                in_ap=table.ap()[s * segrows:
                                                 (s + 1) * segrows, :],
                                idxs_ap=sl_sb[:, iw0 + s * (TSEG * P // 16):
                                              iw0 + (s + 1) * (TSEG * P // 16)],
                                num_idxs=TSEG * P, num_idxs_reg=TSEG * P,
                                elem_size=RW, queue_num=s)
                        # --- window dst rows: static DMA at pid offset
                        xrw = gath.tile([P, RW], f16, tag="xrw", bufs=3)
                        nc.sync.dma_start(
                            out=xrw[:],
                            in_=tv[bass.ds(pid * P, P), w, :])

                        # --- one-hots (DVE)
                        oh = ohp.tile([P, TWS, P], f16, tag="oh")
                        nc.vector.tensor_tensor(
                            out=oh[:],
                            in0=iota_f[:].unsqueeze(1).to_broadcast(
                                [P, TWS, P]),
                            in1=doff_sb[:, w * TWS:(w + 1) * TWS]
                                .unsqueeze(2).to_broadcast([P, TWS, P]),
                            op=ALU.is_equal)
                        ohT = ohp.tile([P, TWS, P], f16, tag="ohT")
                        nc.vector.tensor_scalar(ohT[:], dft[:], iotac_f[:],
                                                None, op0=ALU.is_equal)

                        # --- contiguous xl copy (ACT); frees xg early
                        xls = emath.tile([P, TWS, D], f16, tag="xls",
                                         bufs=3)
                        nc.scalar.copy(out=xls[:], in_=xg[:, :, 0:D])

                        # --- epre = ohT@xr + I@xl accumulated on PE (PSUM)
                        xre = xrep.tile([P, TWS, D], f32_, tag="xre")
                        for t in range(TWS):
                            nc.tensor.matmul(out=xre[:, t, :],
                                             lhsT=ohT[:, t, :],
                                             rhs=xrw[:, D1:D1 + D],
                                             start=True, stop=False)
                            nc.tensor.matmul(out=xre[:, t, :],
                                             lhsT=ident[:],
                                             rhs=xls[:, t, :],
                                             start=False, stop=True)

                        # --- edge softmax math (leaky on ACT, reads PSUM)
                        ee = emath.tile([P, TWS, D], f16, tag="ee", bufs=3)
                        nc.scalar.activation(out=ee[:], in_=xre[:],
                                             func=AF.Prelu, alpha=NEG_ATT)
                        tmp = emath.tile([P, TWS, D], f16, tag="tmp", bufs=3)
                        nc.vector.tensor_tensor(
                            out=tmp[:], in0=ee[:], in1=attrep[:],
                            op=ALU.mult)
                        logits = emath.tile([P, TWS * H], f16, tag="logits")
                        with nc.allow_low_precision(
                                reason="9-16 term f16 logit sums"):
                            nc.vector.tensor_reduce(
                                out=logits[:],
                                in_=tmp[:].rearrange(
                                    "p t (h c) -> p (t h) c", c=C),
                                axis=mybir.AxisListType.X, op=ALU.add)

                        wtx = wtp.tile([P, TWS, D], f16, tag="wtx", bufs=3)
                        nc.scalar.activation(
                            out=wtx[:].rearrange("p t (h c) -> p t h c",
                                                 c=C),
                            in_=logits[:].rearrange("p (t h) -> p t h", h=H)
                            .unsqueeze(3).to_broadcast([P, TWS, H, C]),
                            func=AF.Exp)
                        wt = wtp.tile([P, TWS, OC], f16, tag="wt", bufs=3)
                        nc.scalar.activation(
                            out=wt[:, :, D:OC],
                            in_=logits[:].rearrange("p (t h) -> p t h", h=H),
                            func=AF.Exp)
                        nc.vector.tensor_tensor(out=wt[:, :, 0:D],
                                                in0=xls[:],
                                                in1=wtx[:], op=ALU.mult)

                        # --- scatter into node-major aggregate (PE)
                        # one PSUM bank shared: aggT | t2p | hTp slices
                        bank = aggp.tile([P, 512], f32_, tag="bank")
                        aggT = bank[:, 0:OC]
                        for t in range(TWS):
                            nc.tensor.matmul(out=aggT[:], lhsT=oh[:, t, :],
                                             rhs=wt[:, t, :], start=(t == 0),
                                             stop=(t == TWS - 1))

                        # --- epilogue (node-major)
                        den = epi.tile([P, H], f32_, tag="den")
                        nc.vector.tensor_scalar(den[:], aggT[:, D:OC],
                                                epsP[:], None, op0=ALU.add)
                        rec = epi.tile([P, H], f32_, tag="rec")
                        nc.vector.reciprocal(out=rec[:], in_=den[:])
                        hpre = epi.tile([P, D], f32_, tag="hpre")
                        nc.vector.tensor_tensor(
                            out=hpre[:].rearrange("p (h c) -> p h c", c=C),
                            in0=aggT[:, 0:D].rearrange("p (h c) -> p h c",
                                                       c=C),
                            in1=rec[:].unsqueeze(2).to_broadcast([P, H, C]),
                            op=ALU.mult)

                        if layer == 1:
                            hb = epi.tile([P, D1], f32_, tag="hb")
                            nc.vector.tensor_tensor(out=hb[:], in0=hpre[:],
                                                    in1=b1rep[:], op=ALU.add)
                            h = epi.tile([P, D1], f16, tag="h")
                            nc.scalar.activation(out=h[:], in_=hb[:],
                                                 func=AF.Prelu, alpha=NEG_ACT)
                            hTp = bank[0:D1, 384:512]
                            nc.tensor.matmul(out=hTp[:], lhsT=h[:],
                                             rhs=ident[:], start=True,
                                             stop=True)
                            hT = epi.tile([D1 + 1, P], f16, tag="hT")
                            nc.scalar.copy(out=hT[0:D1, :], in_=hTp[:])
                            nc.vector.memset(hT[D1:D1 + 1, :], 1.0)
                            t2p = bank[:, 128:256]
                            nc.tensor.matmul(out=t2p[:], lhsT=hT[:],
                                             rhs=w2c_sb[:], start=True,
                                             stop=True)
                            st2 = epi.tile([P, RW], f16, tag="st2")
                            nc.scalar.copy(out=st2[:], in_=t2p[:])
                            nc.sync.dma_start(out=t2v[w], in_=st2[:])
                        else:
                            nc.sync.dma_start(
                                out=out_raw.ap()[w * P:(w + 1) * P, :],
                                in_=hpre[:])

            # ---------------- phase 2: layer-1 edges
            edge_layer(1)
            tc.strict_bb_all_engine_barrier()

            # ---------------- phase 3: allgather layer-2 table
            nc.gpsimd.collective_compute(
                "AllGather", mybir.AluOpType.bypass,
                replica_groups=[list(range(cfg.ncores))],
                ins=[t2sh.ap()], outs=[t2g.ap()])
            tc.strict_bb_all_engine_barrier()

            # ---------------- phase 4: layer-2 edges
            edge_layer(2)

    nc.compile()
    return nc


# ---------------------------------------------------------------- host prep
def host_prep(x, edge_index, W1l, b1l, W1r, b1r, att1, bias1,
              W2l, b2l, W2r, b2r, att2, bias2, cfg: Cfg):
    """Numpy-only preprocessing: edge sort/segment/pad + weight layouts."""
    n, e, nsh = cfg.n, cfg.e, cfg.nsh
    src = np.asarray(edge_index[0], dtype=np.int64)
    dst = np.asarray(edge_index[1], dtype=np.int64)

    order = np.argsort(dst, kind="stable")
    src_s, dst_s = src[order], dst[order]
    bounds = np.searchsorted(dst_s, np.arange(cfg.ncores + 1) * nsh)

    def rowmap(g):
        k, loc = g // nsh, g % nsh
        return k * cfg.rows2sh + (loc % P) * cfg.nw + loc // P

    nslots = cfg.nw * SLOTS
    segslots = TSEG * P  # 640
    segrows = cfg.segrows1

    per_core = []
    for k in range(cfg.ncores):
        sl = slice(bounds[k], bounds[k + 1])
        sk, dk = src_s[sl], dst_s[sl]
        dloc = dk - k * nsh
        win = dloc >> 7
        r = rowmap(sk)
        seg = r // segrows
        key = win * SEGS + seg
        o2 = np.argsort(key, kind="stable")
        ks, rs, dls = key[o2], r[o2], dloc[o2]
        gb = np.searchsorted(ks, np.arange(cfg.nw * SEGS + 1))
        cnt = np.diff(gb)
        if cnt.max(initial=0) > segslots:
            return None  # static schedule overflow -> caller falls back
        pos = (ks * segslots + np.arange(len(ks)) - gb[ks]).astype(np.int64)
        idxf = np.zeros(nslots, np.int16)
        off = np.full(nslots, -1.0, np.float16)
        idxf[pos] = (rs - (ks % SEGS) * segrows).astype(np.int16)
        off[pos] = (dls - (ks // SEGS) * WIN).astype(np.float16)

        # wrap idx per (window,segment): [16, 40] blocks, replicate x8
        blocks = idxf.reshape(cfg.nw * SEGS, segslots // 16, 16)
        blocks = blocks.transpose(0, 2, 1)  # [NW*SEGS, 16, 40]
        slw16 = np.ascontiguousarray(
            blocks.transpose(1, 0, 2).reshape(16, -1))  # [16, NW*160]
        core = dict(
            slw=np.tile(slw16, (8, 1)).astype(np.int16),
            doffd=np.ascontiguousarray(off.reshape(cfg.nw * TWS, P).T),
            dofft=np.ascontiguousarray(
                off.reshape(cfg.nw, SLOTS)).astype(np.int8))
        per_core.append(core)

    # xT core-major padded: node (k, loc) at column k*12544 + loc
    xTh = np.zeros((IN + 1, cfg.rows1), np.float16)
    xTh[IN, :] = 1.0
    g = np.arange(n)
    cols = (g // nsh) * cfg.rows2sh + (g % nsh)
    xTh[:IN, cols] = np.asarray(x, np.float32).T.astype(np.float16)
    w1cat = np.concatenate([np.asarray(W1l), np.asarray(W1r)], axis=0)
    w1b = np.concatenate([np.asarray(b1l), np.asarray(b1r)])[None, :]
    w1c_h = np.concatenate([w1cat.T, w1b], axis=0).astype(np.float16)
    w2c_h = np.zeros((D1 + 1, RW), np.float16)
    w2c_h[:D1, 0:D2] = np.asarray(W2l).T
    w2c_h[:D1, D1:D1 + D2] = np.asarray(W2r).T
    w2c_h[D1, 0:D2] = np.asarray(b2l)
    w2c_h[D1, D1:D1 + D2] = np.asarray(b2r)
    att1_h = np.asarray(att1, np.float32).reshape(1, D1).astype(np.float16)
    att2_h = np.asarray(att2, np.float32).reshape(1, D2).astype(np.float16)
    bias1_h = np.asarray(bias1, np.float32).reshape(1, D1)

    shared = dict(xT=xTh, w1c=w1c_h, w2c=w2c_h, att1=att1_h,
                  att2=att2_h, bias1r=bias1_h)
    in_maps = [dict(shared, **pc) for pc in per_core]
    return in_maps


def assemble_output(results, bias2, cfg: Cfg):
    outs = []
    b2 = np.asarray(bias2, np.float32)
    for k in range(cfg.ncores):
        arr = results[k]["out_raw"][:cfg.nsh]  # [nsh, 36] node-major
        outk = arr.reshape(cfg.nsh, H2, OUT).mean(axis=1) + b2[None, :]
        outs.append(outk.astype(np.float32))
    return np.concatenate(outs, axis=0)


# ---------------------------------------------------------------- fallback
def _reference_numpy(x, edge_index, W1l, b1l, W1r, b1r, att1, bias1,
                     W2l, b2l, W2r, b2r, att2, bias2):
    """Pure-numpy fallback (used only if inputs don't fit the static plan)."""
    def gatv2(x, src, dst, Wl, bl, Wr, br, att, bias, concat):
        n = x.shape[0]
        H, C = att.shape
        xl = (x @ Wl.T + bl).reshape(n, H, C)
        xr = (x @ Wr.T + br).reshape(n, H, C)
        ee = xl[src] + xr[dst]
        ee = np.where(ee > 0, ee, NEG_ATT * ee)
        logits = np.einsum("ehc,hc->eh", ee, att)
        m = np.full((n, H), -np.inf, np.float32)
        np.maximum.at(m, dst, logits)
        m = np.where(np.isfinite(m), m, 0.0)
        p = np.exp(logits - m[dst])
        den = np.zeros((n, H), np.float32)
        np.add.at(den, dst, p)
        alpha = p / (den[dst] + 1e-16)
        out = np.zeros((n, H, C), np.float32)
        np.add.at(out, dst, alpha[..., None] * xl[src])
        if concat:
            return out.reshape(n, H * C) + bias
        return out.mean(axis=1) + bias

    src, dst = edge_index[0].astype(np.int64), edge_index[1].astype(np.int64)
    h = gatv2(np.asarray(x, np.float32), src, dst, W1l, b1l, W1r, b1r, att1,
              bias1, True)
    h = np.where(h > 0, h, NEG_ACT * h)
    return gatv2(h, src, dst, W2l, b2l, W2r, b2r, att2, bias2, False)


# ---------------------------------------------------------------- entry point
@functools.lru_cache(maxsize=1)
def _compiled():
    return build_program(CFG)


_LAST_RESULTS = {}


def kernel(x, edge_index, W1l, b1l, W1r, b1r, att1, bias1,
           W2l, b2l, W2r, b2r, att2, bias2):
    args = (x, edge_index, W1l, b1l, W1r, b1r, att1, bias1,
            W2l, b2l, W2r, b2r, att2, bias2)
    if (np.asarray(x).shape != (N, IN)
            or np.asarray(edge_index).shape != (2, E)):
        return _reference_numpy(*[np.asarray(a, np.float32) if i != 1 else
                                  np.asarray(a) for i, a in enumerate(args)])

    in_maps = host_prep(*args, CFG)
    if in_maps is None:
        return _reference_numpy(*[np.asarray(a, np.float32) if i != 1 else
                                  np.asarray(a) for i, a in enumerate(args)])

    from concourse.bass_utils import run_bass_kernel_spmd
    nc = _compiled()
    res = run_bass_kernel_spmd(nc, in_maps, core_ids=list(range(NCORES)),
                               trace=False)
    _LAST_RESULTS["res"] = res
    return assemble_output(res.results, bias2, CFG)


# revision 25
# speedup vs baseline: 1.1487x; 1.0008x over previous
"""GATv2 (2-layer, PyG semantics) on 8 Trainium2 NeuronCores.

Strategy (graph/data parallel, dst-sharded), v2:
  - Nodes sharded by destination range across 8 cores (12500 nodes/core).
  - Feature tables have 256B rows [xl(64,f16) | xr(64,f16)] so the batched
    SWDGE ucode gather (dma_gather, int16 idx, 256B elems) applies. Tables
    split into 4 row segments of 25088 (int16 index range); each 128-dst
    window's edge slots are grouped by src-row segment: 4 segments x 5
    tiles of 128 slots. One dma_gather per (window, segment) replaces the
    per-tile indirect gathers: GpSimd desc-gen cost drops from ~19 x 1.4us
    to ~4 x 1.2us + 1 x 1.0us per window.
  - Both layers' tables use the SAME core-major row mapping
    (row = core*12544 + (loc%128)*98 + loc//128), so each window's 128 dst
    rows sit at a fixed stride-98 pattern: the dst-side window fetch is a
    single static DMA with a partition-id dynamic offset (no SWDGE), and
    the two layers share one set of slot/offset index arrays. A transposed
    one-hot (ohT, from a broadcast dst-offset stream) expands xr to edge
    slots on the tensor engine; the xl+xr add rides the same PSUM
    accumulation.
  - Edge softmax math on DVE/ACT; scatter is one-hot matmuls into a
    node-major PSUM aggregate; node-major epilogue; layer-2 tables
    AllGather once; host finishes mean/bias.
"""

import functools
import sys

import numpy as np

sys.path.insert(0, "/opt/trn_rl_repo")

# ---------------------------------------------------------------- constants
N = 100_000
E = 1_600_000
IN = 9
HID = 16
H1 = 4
H2 = 4
OUT = 9
D1 = H1 * HID  # 64
D2 = H2 * OUT  # 36
NEG_ATT = 0.2
NEG_ACT = 0.01
NCORES = 8
NSH = N // NCORES  # 12500 nodes per core
WIN = 128  # dst nodes per window
P = 128
RW = 128  # table row width (f16): [l 64 | r 64] = 256B
SEGS = 4
TSEG = 5  # tiles (of 128 slots) per segment per window
TWS = SEGS * TSEG  # 20 tiles per window
SLOTS = TWS * P  # 2560 slots per window


class Cfg:
    """Compile-time geometry. Full-size defaults; overridable for sim tests."""

    def __init__(self, n=N, e=E, ncores=NCORES, dense_chunks=28, dfc=7):
        self.n = n
        self.e = e
        self.ncores = ncores
        self.nsh = n // ncores
        self.nw = -(-self.nsh // WIN)  # windows per core (98)
        self.rows1 = ((n + 1023) // 1024) * 1024  # 100352
        self.segrows1 = self.rows1 // SEGS  # 25088
        self.rows2sh = self.nw * WIN  # 12544
        self.rows2 = self.rows2sh * ncores  # 100352
        self.segrows2 = self.rows2 // SEGS
        self.dense_tiles = self.rows1 // P  # 784
        self.dense_chunks = dense_chunks
        assert self.dense_tiles % dense_chunks == 0
        self.chunk_tiles = self.dense_tiles // dense_chunks  # 28
        self.wb = 7 if self.chunk_tiles % 7 == 0 else 1
        assert self.chunk_tiles % self.wb == 0
        self.nb = self.dense_tiles // self.wb  # write batches (112)
        self.dfc = dfc  # windows per stream chunk
        assert self.nw % dfc == 0
        self.nwc = self.nw // dfc
        self.iw = SEGS * (TSEG * P // 16)  # idx cols per window (160)
        assert self.segrows1 <= 32768 and self.segrows2 <= 32768


CFG = Cfg()


# ---------------------------------------------------------------- device code
def build_program(cfg: Cfg):
    """Build the SPMD single-core Bass program (same NEFF on all cores)."""
    import concourse.bacc as bacc
    import concourse.bass as bass
    import concourse.tile as tile
    from concourse import mybir

    f16 = mybir.dt.float16
    i32 = mybir.dt.int32
    i16 = mybir.dt.int16
    i8 = mybir.dt.int8
    AF = mybir.ActivationFunctionType
    ALU = mybir.AluOpType

    nc = bacc.Bacc("TRN2", target_bir_lowering=False, debug=False,
                   num_devices=cfg.ncores, num_swdge_queues=4)

    NW = cfg.nw

    # ---------------- dram I/O
    xT = nc.dram_tensor("xT", [IN + 1, cfg.rows1], f16, kind="ExternalInput")
    w1c = nc.dram_tensor("w1c", [IN + 1, RW], f16, kind="ExternalInput")
    w2c = nc.dram_tensor("w2c", [D1 + 1, RW], f16, kind="ExternalInput")
    f32_ = mybir.dt.float32
    att1 = nc.dram_tensor("att1", [1, D1], f16, kind="ExternalInput")
    att2 = nc.dram_tensor("att2", [1, D2], f16, kind="ExternalInput")
    bias1r = nc.dram_tensor("bias1r", [1, D1], f32_, kind="ExternalInput")
    slw = nc.dram_tensor("slw", [P, NW * cfg.iw], i16, kind="ExternalInput")
    doffd = nc.dram_tensor("doffd", [P, NW * TWS], f16, kind="ExternalInput")
    dofft = nc.dram_tensor("dofft", [NW, SLOTS], i8, kind="ExternalInput")
    out_raw = nc.dram_tensor("out_raw", [cfg.rows2sh, D2], f32_,
                             kind="ExternalOutput")

    t1g = nc.dram_tensor("t1g", [cfg.rows1, RW], f16)
    t2sh = nc.dram_tensor("t2sh", [cfg.rows2sh, RW], f16)
    t2g = nc.dram_tensor("t2g", [cfg.rows2, RW], f16, addr_space="Shared")

    OC1 = D1 + H1  # agg cols layer 1 (num 64 + den 4)
    OC2 = D2 + H2  # agg cols layer 2 (num 36 + den 4)

    with tile.TileContext(nc) as tc:
        import contextlib
        ctx = contextlib.ExitStack()
        with ctx:
            consts = ctx.enter_context(tc.tile_pool(name="consts", bufs=1))
            idxp = ctx.enter_context(tc.tile_pool(name="idxp", bufs=1))
            idxs = ctx.enter_context(tc.tile_pool(name="idxs", bufs=2))
            xtp = ctx.enter_context(tc.tile_pool(name="xtp", bufs=2))
            stage = ctx.enter_context(tc.tile_pool(name="stage", bufs=3))
            gath = ctx.enter_context(tc.tile_pool(name="gath", bufs=2))
            ohp = ctx.enter_context(tc.tile_pool(name="ohp", bufs=2))
            emath = ctx.enter_context(tc.tile_pool(name="emath", bufs=2))
            wtp = ctx.enter_context(tc.tile_pool(name="wtp", bufs=2))
            epi = ctx.enter_context(tc.tile_pool(name="epi", bufs=2))
            # ---------------- constants into SBUF
            w1c_sb = consts.tile([IN + 1, RW], f16)
            nc.sync.dma_start(out=w1c_sb[:], in_=w1c.ap())
            w2c_sb = consts.tile([D1 + 1, RW], f16)
            nc.sync.dma_start(out=w2c_sb[:], in_=w2c.ap())
            b1rep = consts.tile([P, D1], f32_)
            nc.sync.dma_start(out=b1rep[0:1, :], in_=bias1r.ap())
            nc.gpsimd.partition_broadcast(b1rep[:], b1rep[0:1, :])
            att1_sb = consts.tile([P, D1], f16)
            nc.sync.dma_start(out=att1_sb[0:1, :], in_=att1.ap())
            nc.gpsimd.partition_broadcast(att1_sb[:], att1_sb[0:1, :])
            att2_sb = consts.tile([P, D2], f16)
            nc.sync.dma_start(out=att2_sb[0:1, :], in_=att2.ap())
            nc.gpsimd.partition_broadcast(att2_sb[:], att2_sb[0:1, :])
            iota_i = consts.tile([P, P], i32)
            nc.gpsimd.iota(iota_i[:], pattern=[[1, P]], base=0,
                           channel_multiplier=0)
            iota_f = consts.tile([P, P], f16)
            nc.vector.tensor_copy(out=iota_f[:], in_=iota_i[:])
            iotac_i = consts.tile([P, 1], i32)
            nc.gpsimd.iota(iotac_i[:], pattern=[[0, 1]], base=0,
                           channel_multiplier=1)
            iotac_f = consts.tile([P, 1], f32_)
            nc.vector.tensor_copy(out=iotac_f[:], in_=iotac_i[:])
            ident = consts.tile([P, P], f16)
            nc.vector.tensor_scalar(ident[:], iota_f[:], iotac_f[:], None,
                                    op0=ALU.is_equal)
            epsP = consts.tile([P, 1], f32_)
            nc.vector.memset(epsP[:], 1e-16)

            # small idx arrays, resident
            doff_sb = idxp.tile([P, NW * TWS], f16)
            nc.sync.dma_start(out=doff_sb[:], in_=doffd.ap())

            # ---------------- phase 1: dense layer-1 table
            # xT is core-major padded (12544 cols/core); node (k, loc) sits
            # at row k*12544 + (loc%128)*98 + loc//128, so one write batch
            # (wb=7 node-tiles) is 7*256B contiguous per partition.
            ck = cfg.chunk_tiles
            nqb = cfg.nw // cfg.wb
            t1v = t1g.ap().rearrange(
                "(c p qb qw) f -> c qb p (qw f)", c=cfg.ncores, p=P,
                qb=nqb, qw=cfg.wb)
            with tc.tile_pool(name="mmp", bufs=4, space="PSUM") as mmp:
                for c in range(cfg.dense_chunks):
                    xt_sb = xtp.tile([IN + 1, ck * P], f16)
                    nc.sync.dma_start(
                        out=xt_sb[:],
                        in_=xT.ap()[:, c * ck * P:(c + 1) * ck * P])
                    for b in range(ck // cfg.wb):
                        st = stage.tile([P, cfg.wb, RW], f16)
                        for j in range(cfg.wb):
                            t = b * cfg.wb + j
                            mm = mmp.tile([P, RW], f32_)
                            nc.tensor.matmul(
                                out=mm[:], lhsT=xt_sb[:, t * P:(t + 1) * P],
                                rhs=w1c_sb[:], start=True, stop=True)
                            if j % 2 == 0:
                                nc.scalar.copy(out=st[:, j, :], in_=mm[:])
                            else:
                                nc.vector.tensor_copy(out=st[:, j, :],
                                                      in_=mm[:])
                        gb = c * (ck // cfg.wb) + b
                        nc.sync.dma_start(
                            out=t1v[gb // nqb, gb % nqb],
                            in_=st[:].rearrange("p t f -> p (t f)"))

            attrep1 = consts.tile([P, TWS, D1], f16)
            nc.vector.tensor_copy(
                out=attrep1[:],
                in_=att1_sb[:, 0:D1].unsqueeze(1).to_broadcast(
                    [P, TWS, D1]))
            attrep2 = consts.tile([P, TWS, D2], f16)
            nc.vector.tensor_copy(
                out=attrep2[:],
                in_=att2_sb[:, 0:D2].unsqueeze(1).to_broadcast(
                    [P, TWS, D2]))
            pid = nc.partition_id()
            tc.strict_bb_all_engine_barrier()

            xrep = ctx.enter_context(tc.tile_pool(name="xrep", bufs=2,
                                                  space="PSUM"))
            aggp = ctx.enter_context(tc.tile_pool(name="aggp", bufs=2,
                                                  space="PSUM"))

            # ---------------- edge phase builder (shared by both layers)
            def edge_layer(layer):
                if layer == 1:
                    D, H, C, OC = D1, H1, HID, OC1
                    table, segrows, attrep = t1g, cfg.segrows1, attrep1
                else:
                    D, H, C, OC = D2, H2, OUT, OC2
                    table, segrows, attrep = t2g, cfg.segrows2, attrep2

                t2v = t2sh.ap().rearrange("(p w) f -> w p f", p=P, w=NW)
                # window dst rows: c*12544 + p*98 + w -> static strided DMA
                tv = table.ap().rearrange("(cp q) f -> cp q f", q=NW)

                for wc in range(cfg.nwc):
                    # stream this chunk's gather indices
                    sl_sb = idxs.tile([P, cfg.dfc * cfg.iw], i16, tag="sl")
                    nc.sync.dma_start(
                        out=sl_sb[:],
                        in_=slw.ap()[:, wc * cfg.dfc * cfg.iw:
                                     (wc + 1) * cfg.dfc * cfg.iw])
                    for wi in range(cfg.dfc):
                        w = wc * cfg.dfc + wi
                        # --- dst-offset row broadcast (HWDGE, static slice)
                        dft = gath.tile([P, TWS, P], i8, tag="dft", bufs=3)
                        nc.sync.dma_start(
                            out=dft[:].rearrange("p t e -> p (t e)"),
                            in_=dofft.ap()[w:w + 1, :].to_broadcast(
                                [P, SLOTS]))
                        # --- segmented src gathers (SWDGE ucode)
                        xg = gath.tile([P, TWS, RW], f16, tag="xg", bufs=3)
                        iw0 = wi * cfg.iw
                        for s in range(SEGS):
                            nc.gpsimd.dma_gather(
                                out_ap=xg[:, s * TSEG:(s + 1) * TSEG, :],
                                in_ap=table.ap()[s * segrows:
                                                 (s + 1) * segrows, :],
                                idxs_ap=sl_sb[:, iw0 + s * (TSEG * P // 16):
                                              iw0 + (s + 1) * (TSEG * P // 16)],
                                num_idxs=TSEG * P, num_idxs_reg=TSEG * P,
                                elem_size=RW, queue_num=s)
                        # --- window dst rows: static DMA at pid offset
                        xrw = gath.tile([P, RW], f16, tag="xrw", bufs=3)
                        nc.sync.dma_start(
                            out=xrw[:],
                            in_=tv[bass.ds(pid * P, P), w, :])

                        # --- one-hots (DVE)
                        oh = ohp.tile([P, TWS, P], f16, tag="oh")
                        nc.vector.tensor_tensor(
                            out=oh[:],
                            in0=iota_f[:].unsqueeze(1).to_broadcast(
                                [P, TWS, P]),
                            in1=doff_sb[:, w * TWS:(w + 1) * TWS]
                                .unsqueeze(2).to_broadcast([P, TWS, P]),
                            op=ALU.is_equal)
                        ohT = ohp.tile([P, TWS, P], f16, tag="ohT")
                        nc.vector.tensor_scalar(ohT[:], dft[:], iotac_f[:],
                                                None, op0=ALU.is_equal)

                        # --- contiguous xl copy (ACT); frees xg early
                        xls = emath.tile([P, TWS, D], f16, tag="xls",
                                         bufs=3)
                        nc.scalar.copy(out=xls[:], in_=xg[:, :, 0:D])

                        # --- epre = ohT@xr + I@xl accumulated on PE (PSUM)
                        xre = xrep.tile([P, TWS, D], f32_, tag="xre")
                        for t in range(TWS):
                            nc.tensor.matmul(out=xre[:, t, :],
                                             lhsT=ohT[:, t, :],
                                             rhs=xrw[:, D1:D1 + D],
                                             start=True, stop=False)
                            nc.tensor.matmul(out=xre[:, t, :],
                                             lhsT=ident[:],
                                             rhs=xls[:, t, :],
                                             start=False, stop=True)

                        # --- edge softmax math (leaky on ACT, reads PSUM)
                        ee = emath.tile([P, TWS, D], f16, tag="ee", bufs=3)
                        nc.scalar.activation(out=ee[:], in_=xre[:],
                                             func=AF.Prelu, alpha=NEG_ATT)
                        tmp = emath.tile([P, TWS, D], f16, tag="tmp", bufs=3)
                        nc.vector.tensor_tensor(
                            out=tmp[:], in0=ee[:], in1=attrep[:],
                            op=ALU.mult)
                        logits = emath.tile([P, TWS * H], f16, tag="logits")
                        with nc.allow_low_precision(
                                reason="9-16 term f16 logit sums"):
                            nc.vector.tensor_reduce(
                                out=logits[:],
                                in_=tmp[:].rearrange(
                                    "p t (h c) -> p (t h) c", c=C),
                                axis=mybir.AxisListType.X, op=ALU.add)

                        wtx = wtp.tile([P, TWS, D], f16, tag="wtx", bufs=3)
                        nc.scalar.activation(
                            out=wtx[:].rearrange("p t (h c) -> p t h c",
                                                 c=C),
                            in_=logits[:].rearrange("p (t h) -> p t h", h=H)
                            .unsqueeze(3).to_broadcast([P, TWS, H, C]),
                            func=AF.Exp)
                        wt = wtp.tile([P, TWS, OC], f16, tag="wt", bufs=3)
                        nc.scalar.activation(
                            out=wt[:, :, D:OC],
                            in_=logits[:].rearrange("p (t h) -> p t h", h=H),
                            func=AF.Exp)
                        nc.vector.tensor_tensor(out=wt[:, :, 0:D],
                                                in0=xls[:],
                                                in1=wtx[:], op=ALU.mult)

                        # --- scatter into node-major aggregate (PE)
                        # one PSUM bank shared: aggT | t2p | hTp slices
                        bank = aggp.tile([P, 512], f32_, tag="bank")
                        aggT = bank[:, 0:OC]
                        for t in range(TWS):
                            nc.tensor.matmul(out=aggT[:], lhsT=oh[:, t, :],
                                             rhs=wt[:, t, :], start=(t == 0),
                                             stop=(t == TWS - 1))

                        # --- epilogue (node-major)
                        den = epi.tile([P, H], f32_, tag="den")
                        nc.vector.tensor_scalar(den[:], aggT[:, D:OC],
                                                epsP[:], None, op0=ALU.add)
                        rec = epi.tile([P, H], f32_, tag="rec")
                        nc.vector.reciprocal(out=rec[:], in_=den[:])
                        hpre = epi.tile([P, D], f32_, tag="hpre")
                        nc.vector.tensor_tensor(
                            out=hpre[:].rearrange("p (h c) -> p h c", c=C),
                            in0=aggT[:, 0:D].rearrange("p (h c) -> p h c",
                                                       c=C),
                            in1=rec[:].unsqueeze(2).to_broadcast([P, H, C]),
                            op=ALU.mult)

                        if layer == 1:
                            hb = epi.tile([P, D1], f32_, tag="hb")
                            nc.vector.tensor_tensor(out=hb[:], in0=hpre[:],
                                                    in1=b1rep[:], op=ALU.add)
                            h = epi.tile([P, D1], f16, tag="h")
                            nc.scalar.activation(out=h[:], in_=hb[:],
                                                 func=AF.Prelu, alpha=NEG_ACT)
                            hTp = bank[0:D1, 384:512]
                            nc.tensor.matmul(out=hTp[:], lhsT=h[:],
                                             rhs=ident[:], start=True,
                                             stop=True)
                            hT = epi.tile([D1 + 1, P], f16, tag="hT")
                            nc.scalar.copy(out=hT[0:D1, :], in_=hTp[:])
                            nc.vector.memset(hT[D1:D1 + 1, :], 1.0)
                            t2p = bank[:, 128:256]
                            nc.tensor.matmul(out=t2p[:], lhsT=hT[:],
                                             rhs=w2c_sb[:], start=True,
                                             stop=True)
                            st2 = epi.tile([P, RW], f16, tag="st2")
                            nc.scalar.copy(out=st2[:], in_=t2p[:])
                            nc.sync.dma_start(out=t2v[w], in_=st2[:])
                        else:
                            nc.sync.dma_start(
                                out=out_raw.ap()[w * P:(w + 1) * P, :],
                                in_=hpre[:])

            # ---------------- phase 2: layer-1 edges
            edge_layer(1)

            # ---------------- phase 3: allgather layer-2 table
            nc.gpsimd.collective_compute(
                "AllGather", mybir.AluOpType.bypass,
                replica_groups=[list(range(cfg.ncores))],
                ins=[t2sh.ap()], outs=[t2g.ap()])

            # ---------------- phase 4: layer-2 edges
            edge_layer(2)

    nc.compile()
    return nc


# ---------------------------------------------------------------- host prep
def host_prep(x, edge_index, W1l, b1l, W1r, b1r, att1, bias1,
              W2l, b2l, W2r, b2r, att2, bias2, cfg: Cfg):
    """Numpy-only preprocessing: edge sort/segment/pad + weight layouts."""
    n, e, nsh = cfg.n, cfg.e, cfg.nsh
    src = np.asarray(edge_index[0], dtype=np.int64)
    dst = np.asarray(edge_index[1], dtype=np.int64)

    order = np.argsort(dst, kind="stable")
    src_s, dst_s = src[order], dst[order]
    bounds = np.searchsorted(dst_s, np.arange(cfg.ncores + 1) * nsh)

    def rowmap(g):
        k, loc = g // nsh, g % nsh
        return k * cfg.rows2sh + (loc % P) * cfg.nw + loc // P

    nslots = cfg.nw * SLOTS
    segslots = TSEG * P  # 640
    segrows = cfg.segrows1

    per_core = []
    for k in range(cfg.ncores):
        sl = slice(bounds[k], bounds[k + 1])
        sk, dk = src_s[sl], dst_s[sl]
        dloc = dk - k * nsh
        win = dloc >> 7
        r = rowmap(sk)
        seg = r // segrows
        key = win * SEGS + seg
        o2 = np.argsort(key, kind="stable")
        ks, rs, dls = key[o2], r[o2], dloc[o2]
        gb = np.searchsorted(ks, np.arange(cfg.nw * SEGS + 1))
        cnt = np.diff(gb)
        if cnt.max(initial=0) > segslots:
            return None  # static schedule overflow -> caller falls back
        pos = (ks * segslots + np.arange(len(ks)) - gb[ks]).astype(np.int64)
        idxf = np.zeros(nslots, np.int16)
        off = np.full(nslots, -1.0, np.float16)
        idxf[pos] = (rs - (ks % SEGS) * segrows).astype(np.int16)
        off[pos] = (dls - (ks // SEGS) * WIN).astype(np.float16)

        # wrap idx per (window,segment): [16, 40] blocks, replicate x8
        blocks = idxf.reshape(cfg.nw * SEGS, segslots // 16, 16)
        blocks = blocks.transpose(0, 2, 1)  # [NW*SEGS, 16, 40]
        slw16 = np.ascontiguousarray(
            blocks.transpose(1, 0, 2).reshape(16, -1))  # [16, NW*160]
        core = dict(
            slw=np.tile(slw16, (8, 1)).astype(np.int16),
            doffd=np.ascontiguousarray(off.reshape(cfg.nw * TWS, P).T),
            dofft=np.ascontiguousarray(
                off.reshape(cfg.nw, SLOTS)).astype(np.int8))
        per_core.append(core)

    # xT core-major padded: node (k, loc) at column k*12544 + loc
    xTh = np.zeros((IN + 1, cfg.rows1), np.float16)
    xTh[IN, :] = 1.0
    g = np.arange(n)
    cols = (g // nsh) * cfg.rows2sh + (g % nsh)
    xTh[:IN, cols] = np.asarray(x, np.float32).T.astype(np.float16)
    w1cat = np.concatenate([np.asarray(W1l), np.asarray(W1r)], axis=0)
    w1b = np.concatenate([np.asarray(b1l), np.asarray(b1r)])[None, :]
    w1c_h = np.concatenate([w1cat.T, w1b], axis=0).astype(np.float16)
    w2c_h = np.zeros((D1 + 1, RW), np.float16)
    w2c_h[:D1, 0:D2] = np.asarray(W2l).T
    w2c_h[:D1, D1:D1 + D2] = np.asarray(W2r).T
    w2c_h[D1, 0:D2] = np.asarray(b2l)
    w2c_h[D1, D1:D1 + D2] = np.asarray(b2r)
    att1_h = np.asarray(att1, np.float32).reshape(1, D1).astype(np.float16)
    att2_h = np.asarray(att2, np.float32).reshape(1, D2).astype(np.float16)
    bias1_h = np.asarray(bias1, np.float32).reshape(1, D1)

    shared = dict(xT=xTh, w1c=w1c_h, w2c=w2c_h, att1=att1_h,
                  att2=att2_h, bias1r=bias1_h)
    in_maps = [dict(shared, **pc) for pc in per_core]
    return in_maps


def assemble_output(results, bias2, cfg: Cfg):
    outs = []
    b2 = np.asarray(bias2, np.float32)
    for k in range(cfg.ncores):
        arr = results[k]["out_raw"][:cfg.nsh]  # [nsh, 36] node-major
        outk = arr.reshape(cfg.nsh, H2, OUT).mean(axis=1) + b2[None, :]
        outs.append(outk.astype(np.float32))
    return np.concatenate(outs, axis=0)


# ---------------------------------------------------------------- fallback
def _reference_numpy(x, edge_index, W1l, b1l, W1r, b1r, att1, bias1,
                     W2l, b2l, W2r, b2r, att2, bias2):
    """Pure-numpy fallback (used only if inputs don't fit the static plan)."""
    def gatv2(x, src, dst, Wl, bl, Wr, br, att, bias, concat):
        n = x.shape[0]
        H, C = att.shape
        xl = (x @ Wl.T + bl).reshape(n, H, C)
        xr = (x @ Wr.T + br).reshape(n, H, C)
        ee = xl[src] + xr[dst]
        ee = np.where(ee > 0, ee, NEG_ATT * ee)
        logits = np.einsum("ehc,hc->eh", ee, att)
        m = np.full((n, H), -np.inf, np.float32)
        np.maximum.at(m, dst, logits)
        m = np.where(np.isfinite(m), m, 0.0)
        p = np.exp(logits - m[dst])
        den = np.zeros((n, H), np.float32)
        np.add.at(den, dst, p)
        alpha = p / (den[dst] + 1e-16)
        out = np.zeros((n, H, C), np.float32)
        np.add.at(out, dst, alpha[..., None] * xl[src])
        if concat:
            return out.reshape(n, H * C) + bias
        return out.mean(axis=1) + bias

    src, dst = edge_index[0].astype(np.int64), edge_index[1].astype(np.int64)
    h = gatv2(np.asarray(x, np.float32), src, dst, W1l, b1l, W1r, b1r, att1,
              bias1, True)
    h = np.where(h > 0, h, NEG_ACT * h)
    return gatv2(h, src, dst, W2l, b2l, W2r, b2r, att2, bias2, False)


# ---------------------------------------------------------------- entry point
@functools.lru_cache(maxsize=1)
def _compiled():
    return build_program(CFG)


_LAST_RESULTS = {}


def kernel(x, edge_index, W1l, b1l, W1r, b1r, att1, bias1,
           W2l, b2l, W2r, b2r, att2, bias2):
    args = (x, edge_index, W1l, b1l, W1r, b1r, att1, bias1,
            W2l, b2l, W2r, b2r, att2, bias2)
    if (np.asarray(x).shape != (N, IN)
            or np.asarray(edge_index).shape != (2, E)):
        return _reference_numpy(*[np.asarray(a, np.float32) if i != 1 else
                                  np.asarray(a) for i, a in enumerate(args)])

    in_maps = host_prep(*args, CFG)
    if in_maps is None:
        return _reference_numpy(*[np.asarray(a, np.float32) if i != 1 else
                                  np.asarray(a) for i, a in enumerate(args)])

    from concourse.bass_utils import run_bass_kernel_spmd
    nc = _compiled()
    res = run_bass_kernel_spmd(nc, in_maps, core_ids=list(range(NCORES)),
                               trace=False)
    _LAST_RESULTS["res"] = res
    return assemble_output(res.results, bias2, CFG)


# revision 26
# speedup vs baseline: 1.1615x; 1.0111x over previous
"""GATv2 (2-layer, PyG semantics) on 8 Trainium2 NeuronCores.

Strategy (graph/data parallel, dst-sharded), v2:
  - Nodes sharded by destination range across 8 cores (12500 nodes/core).
  - Feature tables have 256B rows [xl(64,f16) | xr(64,f16)] so the batched
    SWDGE ucode gather (dma_gather, int16 idx, 256B elems) applies. Tables
    split into 4 row segments of 25088 (int16 index range); each 128-dst
    window's edge slots are grouped by src-row segment: 4 segments x 5
    tiles of 128 slots. One dma_gather per (window, segment) replaces the
    per-tile indirect gathers: GpSimd desc-gen cost drops from ~19 x 1.4us
    to ~4 x 1.2us + 1 x 1.0us per window.
  - Both layers' tables use the SAME core-major row mapping
    (row = core*12544 + (loc%128)*98 + loc//128), so each window's 128 dst
    rows sit at a fixed stride-98 pattern: the dst-side window fetch is a
    single static DMA with a partition-id dynamic offset (no SWDGE), and
    the two layers share one set of slot/offset index arrays. A transposed
    one-hot (ohT, from a broadcast dst-offset stream) expands xr to edge
    slots on the tensor engine; the xl+xr add rides the same PSUM
    accumulation.
  - Edge softmax math on DVE/ACT; scatter is one-hot matmuls into a
    node-major PSUM aggregate; node-major epilogue; layer-2 tables
    AllGather once; host finishes mean/bias.
"""

import functools
import sys

import numpy as np

sys.path.insert(0, "/opt/trn_rl_repo")

# ---------------------------------------------------------------- constants
N = 100_000
E = 1_600_000
IN = 9
HID = 16
H1 = 4
H2 = 4
OUT = 9
D1 = H1 * HID  # 64
D2 = H2 * OUT  # 36
NEG_ATT = 0.2
NEG_ACT = 0.01
NCORES = 8
NSH = N // NCORES  # 12500 nodes per core
WIN = 128  # dst nodes per window
P = 128
RW = 128  # table row width (f16): [l 64 | r 64] = 256B
SEGS = 4
TSEG = 5  # tiles (of 128 slots) per segment per window
TWS = SEGS * TSEG  # 20 tiles per window
SLOTS = TWS * P  # 2560 slots per window


class Cfg:
    """Compile-time geometry. Full-size defaults; overridable for sim tests."""

    def __init__(self, n=N, e=E, ncores=NCORES, dense_chunks=28, dfc=7):
        self.n = n
        self.e = e
        self.ncores = ncores
        self.nsh = n // ncores
        self.nw = -(-self.nsh // WIN)  # windows per core (98)
        self.rows1 = ((n + 1023) // 1024) * 1024  # 100352
        self.segrows1 = self.rows1 // SEGS  # 25088
        self.rows2sh = self.nw * WIN  # 12544
        self.rows2 = self.rows2sh * ncores  # 100352
        self.segrows2 = self.rows2 // SEGS
        self.dense_tiles = self.rows1 // P  # 784
        self.dense_chunks = dense_chunks
        assert self.dense_tiles % dense_chunks == 0
        self.chunk_tiles = self.dense_tiles // dense_chunks  # 28
        self.wb = 7 if self.chunk_tiles % 7 == 0 else 1
        assert self.chunk_tiles % self.wb == 0
        self.nb = self.dense_tiles // self.wb  # write batches (112)
        self.dfc = dfc  # windows per stream chunk
        assert self.nw % dfc == 0
        self.nwc = self.nw // dfc
        self.iw = SEGS * (TSEG * P // 16)  # idx cols per window (160)
        assert self.segrows1 <= 32768 and self.segrows2 <= 32768


CFG = Cfg()


# ---------------------------------------------------------------- device code
def build_program(cfg: Cfg):
    """Build the SPMD single-core Bass program (same NEFF on all cores)."""
    import concourse.bacc as bacc
    import concourse.bass as bass
    import concourse.tile as tile
    from concourse import mybir

    f16 = mybir.dt.float16
    i32 = mybir.dt.int32
    i16 = mybir.dt.int16
    i8 = mybir.dt.int8
    AF = mybir.ActivationFunctionType
    ALU = mybir.AluOpType

    nc = bacc.Bacc("TRN2", target_bir_lowering=False, debug=False,
                   num_devices=cfg.ncores, num_swdge_queues=4)

    NW = cfg.nw

    # ---------------- dram I/O
    xT = nc.dram_tensor("xT", [IN + 1, cfg.rows1], f16, kind="ExternalInput")
    w1c = nc.dram_tensor("w1c", [IN + 1, RW], f16, kind="ExternalInput")
    w2c = nc.dram_tensor("w2c", [D1 + 1, RW], f16, kind="ExternalInput")
    f32_ = mybir.dt.float32
    att1 = nc.dram_tensor("att1", [1, D1], f16, kind="ExternalInput")
    att2 = nc.dram_tensor("att2", [1, D2], f16, kind="ExternalInput")
    bias1r = nc.dram_tensor("bias1r", [1, D1], f32_, kind="ExternalInput")
    slw = nc.dram_tensor("slw", [P, NW * cfg.iw], i16, kind="ExternalInput")
    doffd = nc.dram_tensor("doffd", [P, NW * TWS], f16, kind="ExternalInput")
    dofft = nc.dram_tensor("dofft", [NW, SLOTS], i8, kind="ExternalInput")
    out_raw = nc.dram_tensor("out_raw", [cfg.rows2sh, D2], f32_,
                             kind="ExternalOutput")

    t1g = nc.dram_tensor("t1g", [cfg.rows1, RW], f16)
    t2sh = nc.dram_tensor("t2sh", [cfg.rows2sh, RW], f16)
    t2g = nc.dram_tensor("t2g", [cfg.rows2, RW], f16, addr_space="Shared")

    OC1 = D1 + H1  # agg cols layer 1 (num 64 + den 4)
    OC2 = D2 + H2  # agg cols layer 2 (num 36 + den 4)

    with tile.TileContext(nc) as tc:
        import contextlib
        ctx = contextlib.ExitStack()
        with ctx:
            consts = ctx.enter_context(tc.tile_pool(name="consts", bufs=1))
            idxp = ctx.enter_context(tc.tile_pool(name="idxp", bufs=1))
            idxs = ctx.enter_context(tc.tile_pool(name="idxs", bufs=2))
            xtp = ctx.enter_context(tc.tile_pool(name="xtp", bufs=2))
            stage = ctx.enter_context(tc.tile_pool(name="stage", bufs=3))
            gath = ctx.enter_context(tc.tile_pool(name="gath", bufs=2))
            ohp = ctx.enter_context(tc.tile_pool(name="ohp", bufs=2))
            emath = ctx.enter_context(tc.tile_pool(name="emath", bufs=2))
            wtp = ctx.enter_context(tc.tile_pool(name="wtp", bufs=2))
            epi = ctx.enter_context(tc.tile_pool(name="epi", bufs=2))
            # ---------------- constants into SBUF
            w1c_sb = consts.tile([IN + 1, RW], f16)
            nc.sync.dma_start(out=w1c_sb[:], in_=w1c.ap())
            w2c_sb = consts.tile([D1 + 1, RW], f16)
            nc.sync.dma_start(out=w2c_sb[:], in_=w2c.ap())
            b1rep = consts.tile([P, D1], f32_)
            nc.sync.dma_start(out=b1rep[0:1, :], in_=bias1r.ap())
            nc.gpsimd.partition_broadcast(b1rep[:], b1rep[0:1, :])
            att1_sb = consts.tile([P, D1], f16)
            nc.sync.dma_start(out=att1_sb[0:1, :], in_=att1.ap())
            nc.gpsimd.partition_broadcast(att1_sb[:], att1_sb[0:1, :])
            att2_sb = consts.tile([P, D2], f16)
            nc.sync.dma_start(out=att2_sb[0:1, :], in_=att2.ap())
            nc.gpsimd.partition_broadcast(att2_sb[:], att2_sb[0:1, :])
            iota_i = consts.tile([P, P], i32)
            nc.gpsimd.iota(iota_i[:], pattern=[[1, P]], base=0,
                           channel_multiplier=0)
            iota_f = consts.tile([P, P], f16)
            nc.vector.tensor_copy(out=iota_f[:], in_=iota_i[:])
            iotac_i = consts.tile([P, 1], i32)
            nc.gpsimd.iota(iotac_i[:], pattern=[[0, 1]], base=0,
                           channel_multiplier=1)
            iotac_f = consts.tile([P, 1], f32_)
            nc.vector.tensor_copy(out=iotac_f[:], in_=iotac_i[:])
            ident = consts.tile([P, P], f16)
            nc.vector.tensor_scalar(ident[:], iota_f[:], iotac_f[:], None,
                                    op0=ALU.is_equal)
            epsP = consts.tile([P, 1], f32_)
            nc.vector.memset(epsP[:], 1e-16)

            # small idx arrays, resident
            doff_sb = idxp.tile([P, NW * TWS], f16)
            nc.sync.dma_start(out=doff_sb[:], in_=doffd.ap())

            # ---------------- phase 1: dense layer-1 table
            # xT is core-major padded (12544 cols/core); node (k, loc) sits
            # at row k*12544 + (loc%128)*98 + loc//128, so one write batch
            # (wb=7 node-tiles) is 7*256B contiguous per partition.
            ck = cfg.chunk_tiles
            nqb = cfg.nw // cfg.wb
            t1v = t1g.ap().rearrange(
                "(c p qb qw) f -> c qb p (qw f)", c=cfg.ncores, p=P,
                qb=nqb, qw=cfg.wb)
            with tc.tile_pool(name="mmp", bufs=4, space="PSUM") as mmp:
                for c in range(cfg.dense_chunks):
                    xt_sb = xtp.tile([IN + 1, ck * P], f16)
                    nc.sync.dma_start(
                        out=xt_sb[:],
                        in_=xT.ap()[:, c * ck * P:(c + 1) * ck * P])
                    for b in range(ck // cfg.wb):
                        st = stage.tile([P, cfg.wb, RW], f16)
                        for j in range(cfg.wb):
                            t = b * cfg.wb + j
                            mm = mmp.tile([P, RW], f32_)
                            nc.tensor.matmul(
                                out=mm[:], lhsT=xt_sb[:, t * P:(t + 1) * P],
                                rhs=w1c_sb[:], start=True, stop=True)
                            if j % 2 == 0:
                                nc.scalar.copy(out=st[:, j, :], in_=mm[:])
                            else:
                                nc.vector.tensor_copy(out=st[:, j, :],
                                                      in_=mm[:])
                        gb = c * (ck // cfg.wb) + b
                        nc.sync.dma_start(
                            out=t1v[gb // nqb, gb % nqb],
                            in_=st[:].rearrange("p t f -> p (t f)"))

            attrep1 = consts.tile([P, TWS, D1], f16)
            nc.vector.tensor_copy(
                out=attrep1[:],
                in_=att1_sb[:, 0:D1].unsqueeze(1).to_broadcast(
                    [P, TWS, D1]))
            attrep2 = consts.tile([P, TWS, D2], f16)
            nc.vector.tensor_copy(
                out=attrep2[:],
                in_=att2_sb[:, 0:D2].unsqueeze(1).to_broadcast(
                    [P, TWS, D2]))
            pid = nc.partition_id()

            xrep = ctx.enter_context(tc.tile_pool(name="xrep", bufs=2,
                                                  space="PSUM"))
            aggp = ctx.enter_context(tc.tile_pool(name="aggp", bufs=2,
                                                  space="PSUM"))

            # ---------------- edge phase builder (shared by both layers)
            def edge_layer(layer):
                if layer == 1:
                    D, H, C, OC = D1, H1, HID, OC1
                    table, segrows, attrep = t1g, cfg.segrows1, attrep1
                else:
                    D, H, C, OC = D2, H2, OUT, OC2
                    table, segrows, attrep = t2g, cfg.segrows2, attrep2

                t2v = t2sh.ap().rearrange("(p w) f -> w p f", p=P, w=NW)
                # window dst rows: c*12544 + p*98 + w -> static strided DMA
                tv = table.ap().rearrange("(cp q) f -> cp q f", q=NW)

                for wc in range(cfg.nwc):
                    # stream this chunk's gather indices
                    sl_sb = idxs.tile([P, cfg.dfc * cfg.iw], i16, tag="sl")
                    nc.sync.dma_start(
                        out=sl_sb[:],
                        in_=slw.ap()[:, wc * cfg.dfc * cfg.iw:
                                     (wc + 1) * cfg.dfc * cfg.iw])
                    for wi in range(cfg.dfc):
                        w = wc * cfg.dfc + wi
                        # --- dst-offset row broadcast (HWDGE, static slice)
                        dft = gath.tile([P, TWS, P], i8, tag="dft", bufs=3)
                        nc.sync.dma_start(
                            out=dft[:].rearrange("p t e -> p (t e)"),
                            in_=dofft.ap()[w:w + 1, :].to_broadcast(
                                [P, SLOTS]))
                        # --- segmented src gathers (SWDGE ucode)
                        xg = gath.tile([P, TWS, RW], f16, tag="xg", bufs=3)
                        iw0 = wi * cfg.iw
                        for s in range(SEGS):
                            nc.gpsimd.dma_gather(
                                out_ap=xg[:, s * TSEG:(s + 1) * TSEG, :],
                                in_ap=table.ap()[s * segrows:
                                                 (s + 1) * segrows, :],
                                idxs_ap=sl_sb[:, iw0 + s * (TSEG * P // 16):
                                              iw0 + (s + 1) * (TSEG * P // 16)],
                                num_idxs=TSEG * P, num_idxs_reg=TSEG * P,
                                elem_size=RW, queue_num=s)
                        # --- window dst rows: static DMA at pid offset
                        xrw = gath.tile([P, RW], f16, tag="xrw", bufs=3)
                        nc.sync.dma_start(
                            out=xrw[:],
                            in_=tv[bass.ds(pid * P, P), w, :])

                        # --- one-hots (DVE)
                        oh = ohp.tile([P, TWS, P], f16, tag="oh")
                        nc.vector.tensor_tensor(
                            out=oh[:],
                            in0=iota_f[:].unsqueeze(1).to_broadcast(
                                [P, TWS, P]),
                            in1=doff_sb[:, w * TWS:(w + 1) * TWS]
                                .unsqueeze(2).to_broadcast([P, TWS, P]),
                            op=ALU.is_equal)
                        ohT = ohp.tile([P, TWS, P], f16, tag="ohT")
                        nc.vector.tensor_scalar(ohT[:], dft[:], iotac_f[:],
                                                None, op0=ALU.is_equal)

                        # --- contiguous xl copy (ACT); frees xg early
                        xls = emath.tile([P, TWS, D], f16, tag="xls",
                                         bufs=3)
                        nc.scalar.copy(out=xls[:], in_=xg[:, :, 0:D])

                        # --- epre = ohT@xr + I@xl accumulated on PE (PSUM)
                        xre = xrep.tile([P, TWS, D], f32_, tag="xre")
                        for t in range(TWS):
                            nc.tensor.matmul(out=xre[:, t, :],
                                             lhsT=ohT[:, t, :],
                                             rhs=xrw[:, D1:D1 + D],
                                             start=True, stop=False)
                            nc.tensor.matmul(out=xre[:, t, :],
                                             lhsT=ident[:],
                                             rhs=xls[:, t, :],
                                             start=False, stop=True)

                        # --- edge softmax math (leaky on ACT, reads PSUM)
                        ee = emath.tile([P, TWS, D], f16, tag="ee", bufs=3)
                        nc.scalar.activation(out=ee[:], in_=xre[:],
                                             func=AF.Prelu, alpha=NEG_ATT)
                        tmp = emath.tile([P, TWS, D], f16, tag="tmp", bufs=3)
                        nc.vector.tensor_tensor(
                            out=tmp[:], in0=ee[:], in1=attrep[:],
                            op=ALU.mult)
                        logits = emath.tile([P, TWS * H], f16, tag="logits")
                        with nc.allow_low_precision(
                                reason="9-16 term f16 logit sums"):
                            nc.vector.tensor_reduce(
                                out=logits[:],
                                in_=tmp[:].rearrange(
                                    "p t (h c) -> p (t h) c", c=C),
                                axis=mybir.AxisListType.X, op=ALU.add)

                        wtx = wtp.tile([P, TWS, D], f16, tag="wtx", bufs=3)
                        nc.scalar.activation(
                            out=wtx[:].rearrange("p t (h c) -> p t h c",
                                                 c=C),
                            in_=logits[:].rearrange("p (t h) -> p t h", h=H)
                            .unsqueeze(3).to_broadcast([P, TWS, H, C]),
                            func=AF.Exp)
                        wt = wtp.tile([P, TWS, OC], f16, tag="wt", bufs=3)
                        nc.scalar.activation(
                            out=wt[:, :, D:OC],
                            in_=logits[:].rearrange("p (t h) -> p t h", h=H),
                            func=AF.Exp)
                        nc.vector.tensor_tensor(out=wt[:, :, 0:D],
                                                in0=xls[:],
                                                in1=wtx[:], op=ALU.mult)

                        # --- scatter into node-major aggregate (PE)
                        # one PSUM bank shared: aggT | t2p | hTp slices
                        bank = aggp.tile([P, 512], f32_, tag="bank")
                        aggT = bank[:, 0:OC]
                        for t in range(TWS):
                            nc.tensor.matmul(out=aggT[:], lhsT=oh[:, t, :],
                                             rhs=wt[:, t, :], start=(t == 0),
                                             stop=(t == TWS - 1))

                        # --- epilogue (node-major)
                        den = epi.tile([P, H], f32_, tag="den")
                        nc.vector.tensor_scalar(den[:], aggT[:, D:OC],
                                                epsP[:], None, op0=ALU.add)
                        rec = epi.tile([P, H], f32_, tag="rec")
                        nc.vector.reciprocal(out=rec[:], in_=den[:])
                        hpre = epi.tile([P, D], f32_, tag="hpre")
                        nc.vector.tensor_tensor(
                            out=hpre[:].rearrange("p (h c) -> p h c", c=C),
                            in0=aggT[:, 0:D].rearrange("p (h c) -> p h c",
                                                       c=C),
                            in1=rec[:].unsqueeze(2).to_broadcast([P, H, C]),
                            op=ALU.mult)

                        if layer == 1:
                            hb = epi.tile([P, D1], f32_, tag="hb")
                            nc.vector.tensor_tensor(out=hb[:], in0=hpre[:],
                                                    in1=b1rep[:], op=ALU.add)
                            h = epi.tile([P, D1], f16, tag="h")
                            nc.scalar.activation(out=h[:], in_=hb[:],
                                                 func=AF.Prelu, alpha=NEG_ACT)
                            hTp = bank[0:D1, 384:512]
                            nc.tensor.matmul(out=hTp[:], lhsT=h[:],
                                             rhs=ident[:], start=True,
                                             stop=True)
                            hT = epi.tile([D1 + 1, P], f16, tag="hT")
                            nc.scalar.copy(out=hT[0:D1, :], in_=hTp[:])
                            nc.vector.memset(hT[D1:D1 + 1, :], 1.0)
                            t2p = bank[:, 128:256]
                            nc.tensor.matmul(out=t2p[:], lhsT=hT[:],
                                             rhs=w2c_sb[:], start=True,
                                             stop=True)
                            st2 = epi.tile([P, RW], f16, tag="st2")
                            nc.scalar.copy(out=st2[:], in_=t2p[:])
                            nc.sync.dma_start(out=t2v[w], in_=st2[:])
                        else:
                            nc.sync.dma_start(
                                out=out_raw.ap()[w * P:(w + 1) * P, :],
                                in_=hpre[:])

            # ---------------- phase 2: layer-1 edges
            edge_layer(1)

            # ---------------- phase 3: allgather layer-2 table
            nc.gpsimd.collective_compute(
                "AllGather", mybir.AluOpType.bypass,
                replica_groups=[list(range(cfg.ncores))],
                ins=[t2sh.ap()], outs=[t2g.ap()])

            # ---------------- phase 4: layer-2 edges
            edge_layer(2)

    nc.compile()
    return nc


# ---------------------------------------------------------------- host prep
def host_prep(x, edge_index, W1l, b1l, W1r, b1r, att1, bias1,
              W2l, b2l, W2r, b2r, att2, bias2, cfg: Cfg):
    """Numpy-only preprocessing: edge sort/segment/pad + weight layouts."""
    n, e, nsh = cfg.n, cfg.e, cfg.nsh
    src = np.asarray(edge_index[0], dtype=np.int64)
    dst = np.asarray(edge_index[1], dtype=np.int64)

    order = np.argsort(dst, kind="stable")
    src_s, dst_s = src[order], dst[order]
    bounds = np.searchsorted(dst_s, np.arange(cfg.ncores + 1) * nsh)

    def rowmap(g):
        k, loc = g // nsh, g % nsh
        return k * cfg.rows2sh + (loc % P) * cfg.nw + loc // P

    nslots = cfg.nw * SLOTS
    segslots = TSEG * P  # 640
    segrows = cfg.segrows1

    per_core = []
    for k in range(cfg.ncores):
        sl = slice(bounds[k], bounds[k + 1])
        sk, dk = src_s[sl], dst_s[sl]
        dloc = dk - k * nsh
        win = dloc >> 7
        r = rowmap(sk)
        seg = r // segrows
        key = win * SEGS + seg
        o2 = np.argsort(key, kind="stable")
        ks, rs, dls = key[o2], r[o2], dloc[o2]
        gb = np.searchsorted(ks, np.arange(cfg.nw * SEGS + 1))
        cnt = np.diff(gb)
        if cnt.max(initial=0) > segslots:
            return None  # static schedule overflow -> caller falls back
        pos = (ks * segslots + np.arange(len(ks)) - gb[ks]).astype(np.int64)
        idxf = np.zeros(nslots, np.int16)
        off = np.full(nslots, -1.0, np.float16)
        idxf[pos] = (rs - (ks % SEGS) * segrows).astype(np.int16)
        off[pos] = (dls - (ks // SEGS) * WIN).astype(np.float16)

        # wrap idx per (window,segment): [16, 40] blocks, replicate x8
        blocks = idxf.reshape(cfg.nw * SEGS, segslots // 16, 16)
        blocks = blocks.transpose(0, 2, 1)  # [NW*SEGS, 16, 40]
        slw16 = np.ascontiguousarray(
            blocks.transpose(1, 0, 2).reshape(16, -1))  # [16, NW*160]
        core = dict(
            slw=np.tile(slw16, (8, 1)).astype(np.int16),
            doffd=np.ascontiguousarray(off.reshape(cfg.nw * TWS, P).T),
            dofft=np.ascontiguousarray(
                off.reshape(cfg.nw, SLOTS)).astype(np.int8))
        per_core.append(core)

    # xT core-major padded: node (k, loc) at column k*12544 + loc
    xTh = np.zeros((IN + 1, cfg.rows1), np.float16)
    xTh[IN, :] = 1.0
    g = np.arange(n)
    cols = (g // nsh) * cfg.rows2sh + (g % nsh)
    xTh[:IN, cols] = np.asarray(x, np.float32).T.astype(np.float16)
    w1cat = np.concatenate([np.asarray(W1l), np.asarray(W1r)], axis=0)
    w1b = np.concatenate([np.asarray(b1l), np.asarray(b1r)])[None, :]
    w1c_h = np.concatenate([w1cat.T, w1b], axis=0).astype(np.float16)
    w2c_h = np.zeros((D1 + 1, RW), np.float16)
    w2c_h[:D1, 0:D2] = np.asarray(W2l).T
    w2c_h[:D1, D1:D1 + D2] = np.asarray(W2r).T
    w2c_h[D1, 0:D2] = np.asarray(b2l)
    w2c_h[D1, D1:D1 + D2] = np.asarray(b2r)
    att1_h = np.asarray(att1, np.float32).reshape(1, D1).astype(np.float16)
    att2_h = np.asarray(att2, np.float32).reshape(1, D2).astype(np.float16)
    bias1_h = np.asarray(bias1, np.float32).reshape(1, D1)

    shared = dict(xT=xTh, w1c=w1c_h, w2c=w2c_h, att1=att1_h,
                  att2=att2_h, bias1r=bias1_h)
    in_maps = [dict(shared, **pc) for pc in per_core]
    return in_maps


def assemble_output(results, bias2, cfg: Cfg):
    outs = []
    b2 = np.asarray(bias2, np.float32)
    for k in range(cfg.ncores):
        arr = results[k]["out_raw"][:cfg.nsh]  # [nsh, 36] node-major
        outk = arr.reshape(cfg.nsh, H2, OUT).mean(axis=1) + b2[None, :]
        outs.append(outk.astype(np.float32))
    return np.concatenate(outs, axis=0)


# ---------------------------------------------------------------- fallback
def _reference_numpy(x, edge_index, W1l, b1l, W1r, b1r, att1, bias1,
                     W2l, b2l, W2r, b2r, att2, bias2):
    """Pure-numpy fallback (used only if inputs don't fit the static plan)."""
    def gatv2(x, src, dst, Wl, bl, Wr, br, att, bias, concat):
        n = x.shape[0]
        H, C = att.shape
        xl = (x @ Wl.T + bl).reshape(n, H, C)
        xr = (x @ Wr.T + br).reshape(n, H, C)
        ee = xl[src] + xr[dst]
        ee = np.where(ee > 0, ee, NEG_ATT * ee)
        logits = np.einsum("ehc,hc->eh", ee, att)
        m = np.full((n, H), -np.inf, np.float32)
        np.maximum.at(m, dst, logits)
        m = np.where(np.isfinite(m), m, 0.0)
        p = np.exp(logits - m[dst])
        den = np.zeros((n, H), np.float32)
        np.add.at(den, dst, p)
        alpha = p / (den[dst] + 1e-16)
        out = np.zeros((n, H, C), np.float32)
        np.add.at(out, dst, alpha[..., None] * xl[src])
        if concat:
            return out.reshape(n, H * C) + bias
        return out.mean(axis=1) + bias

    src, dst = edge_index[0].astype(np.int64), edge_index[1].astype(np.int64)
    h = gatv2(np.asarray(x, np.float32), src, dst, W1l, b1l, W1r, b1r, att1,
              bias1, True)
    h = np.where(h > 0, h, NEG_ACT * h)
    return gatv2(h, src, dst, W2l, b2l, W2r, b2r, att2, bias2, False)


# ---------------------------------------------------------------- entry point
@functools.lru_cache(maxsize=1)
def _compiled():
    return build_program(CFG)


_LAST_RESULTS = {}


def kernel(x, edge_index, W1l, b1l, W1r, b1r, att1, bias1,
           W2l, b2l, W2r, b2r, att2, bias2):
    args = (x, edge_index, W1l, b1l, W1r, b1r, att1, bias1,
            W2l, b2l, W2r, b2r, att2, bias2)
    if (np.asarray(x).shape != (N, IN)
            or np.asarray(edge_index).shape != (2, E)):
        return _reference_numpy(*[np.asarray(a, np.float32) if i != 1 else
                                  np.asarray(a) for i, a in enumerate(args)])

    in_maps = host_prep(*args, CFG)
    if in_maps is None:
        return _reference_numpy(*[np.asarray(a, np.float32) if i != 1 else
                                  np.asarray(a) for i, a in enumerate(args)])

    from concourse.bass_utils import run_bass_kernel_spmd
    nc = _compiled()
    res = run_bass_kernel_spmd(nc, in_maps, core_ids=list(range(NCORES)),
                               trace=False)
    _LAST_RESULTS["res"] = res
    return assemble_output(res.results, bias2, CFG)
